# revision 14
# baseline (speedup 1.0000x reference)
"""Trainium2 Bass kernel for a 4-head attention layer with post-softmax
affine blend (attn = 0.5*softmax(qk/sqrt(dh)) + 0.5), distributed over 8
NeuronCores.

Reference computation (B=2, S=4096, D=128, H=4, Dh=32):
    k = einsum('ihd,bpd->biph', W_K, x)
    q = einsum('ihd,bpd->biph', W_Q, x)
    v = einsum('ihd,bpd->biph', W_V, x)
    scores = einsum('biph,biqh->biqp', k, q) / sqrt(32)
    attn   = softmax(scores, -1) * 0.5 + 0.5
    z      = einsum('biph,biqp->biqh', v, attn)
    out    = einsum('df,bpf->bpd', W_O, z_flat)

Sharding: 8 cores = (batch b in {0,1}) x (query chunk qc in 4 x 1024).
Each core computes all 4 heads for its 1024 queries against all 4096
keys and emits the disjoint output slice out[b, qc*1024:(qc+1)*1024, :]
(transposed on-chip as [D, 1024]; the host unshard transposes back).

Per-core algorithm (everything stays on-chip):
  - Prologue: batched input DMAs split over two queues, then a dense
    projection burst (k^T/q^T/v on TensorE) that also un-throttles the
    PE HAM clock gate.
  - Main rounds (2 q-halves x 32 key-blocks): scores^T tiles
    [128 keys x 2x512 q] via 4 row-packed K=32 matmuls into a 3-deep
    PSUM ring; exp is computed from PSUM split across ScalarE
    (activation, scale folded into W_Q) and VectorE (Schraudolph exp2
    custom DVE op writing bf16 bits via int16); attn@v accumulates
    z^T in PSUM with col-packed M=64 matmuls whose stationary operand
    [v_i | ones | 0] also accumulates the softmax denominator.
  - Epilogue per q-half: denominators are reshaped through DRAM onto
    128 partitions for a cheap VectorE reciprocal, broadcast back, and
    applied to z^T.
  - Final projection is transposed (W_O slices stationary, z^T moving,
    N=512): out^T accumulates in PSUM; the uniform 0.5*sum_k(v) blend
    constant is added via two K=1 bf16 hi/lo matmuls.
"""

import math

import numpy as np
import ml_dtypes

BF16 = ml_dtypes.bfloat16

B, S, D, H, DH = 2, 4096, 128, 4, 32
QCHUNK = 1024  # queries per core
NCORES = 8
NKB = S // 128  # 32 key blocks
# exp(s) is computed as exp((s * 2^15 * log2(e)) * ln(2) / 2^15); the big
# pre-scale is folded into W_Q so a bit-trick exp2 on VectorE can share the
# same score tensor.
PRESCALE = (2.0**15) * math.log2(math.e) / math.sqrt(DH)
ACT_SCALE = math.log(2.0) / (2.0**15)

# Schraudolph exp2 constant: sigma balances the multiplicative error of the
# linear-mantissa approximation; folded into the int16 bf16-bit construction.
EXP2_SIGMA = 0.02979

_PROGRAM = None


def _register_exp2():
    """Register (once) a fused y = x*C0 + C1 custom DVE op whose int16
    output, reinterpreted as bf16, is 2^(x/2^15) a la Schraudolph."""
    from concourse import dve_ops
    from concourse.dve_spec import Spec, Src0, C0, C1, lower, _has_src1
    from concourse.dve_uop import DveOpSpec

    name = "EXP2_SCHRAU_ANT"
    for o in dve_ops.OPS:
        if o.name == name:
            return o
    spec = Spec(body=Src0 * C0 + C1,
                reference=lambda in0, in1, c0, c1, c2: in0 * c0 + c1)
    opcode = dve_ops._CUSTOM_DVE_ROW_BASE + len(dve_ops.OPS)
    shas = {}
    for ver in ("v3", "v4"):
        s = DveOpSpec(name=name, opcode=opcode, uops=lower(spec, ver=ver),
                      rd1_en=_has_src1(spec))
        shas[ver] = s.sha(ver)
    op = dve_ops.DveOp(name, spec, subdim=False, uops_sha=shas)
    dve_ops.OPS.append(op)
    dve_ops.CUSTOM_DVE_SPECS[name] = spec
    dve_ops._SUB_OPCODE_FOR_NAME[name] = opcode
    return op


def _build_program(loop_n: int = 1, ve_hi: bool = True, flip_every: int = 10):
    import concourse.bass as bass
    import concourse.mybir as mybir
    import concourse.tile as tile
    from concourse import bacc
    from contextlib import ExitStack

    import dataclasses

    f32 = mybir.dt.float32
    bf16 = mybir.dt.bfloat16
    AF = mybir.ActivationFunctionType
    exp2_op = _register_exp2()

    def i16_alias(ap):
        h = dataclasses.replace(ap.tensor, dtype=mybir.dt.int16)
        return bass.AP(tensor=h, offset=ap.offset, ap=[list(d) for d in ap.ap])

    def bf16_hi_alias(ap):
        """View an f32 [P, N] AP as the bf16 high halves: [P, N] bf16,
        element stride 2, offset +1 (little-endian high 2 bytes)."""
        h = dataclasses.replace(
            ap.tensor, dtype=mybir.dt.bfloat16,
            shape=[ap.tensor.shape[0], ap.tensor.shape[1] * 2],
        )
        newap = [[ap.ap[0][0] * 2, ap.ap[0][1]]] + [
            [d[0] * 2, d[1]] for d in ap.ap[1:]
        ]
        return bass.AP(tensor=h, offset=ap.offset * 2 + 1, ap=newap)

    def pstride(tile_ap, row0, step, n, col0, ncols):
        """Partition-strided view of a [128, C] tile: rows row0, row0+step,
        ... (n of them), cols col0:col0+ncols."""
        rs = tile_ap.ap[0][0]
        cs = tile_ap.ap[1][0]
        return bass.AP(
            tensor=tile_ap.tensor,
            offset=tile_ap.offset + row0 * rs + col0 * cs,
            ap=[[rs * step, n], [cs, ncols]],
        )

    nc = bacc.Bacc(None, target_bir_lowering=False)

    xkT = nc.dram_tensor("xkT", [D, S], bf16, kind="ExternalInput")
    xqT = nc.dram_tensor("xqT", [D, QCHUNK], bf16, kind="ExternalInput")
    # wpack: cols 0:128 wq (pre-scaled), 128:256 wk, 256:384 wv
    wpack = nc.dram_tensor("wpack", [D, 3 * 128], bf16, kind="ExternalInput")
    # woT[p, r, :]: rows 0:32 head 2p, rows 64:96 head 2p+1 (0.5*W_O.T slices)
    woT = nc.dram_tensor("woT", [2, 128, D], bf16, kind="ExternalInput")
    # cpack: cols 0:128 bf16 hi of blend constant c, 128:256 lo residual
    cpack = nc.dram_tensor("cpack", [1, 2 * D], bf16, kind="ExternalInput")
    out = nc.dram_tensor("out", [D, QCHUNK], f32, kind="ExternalOutput")

    with tile.TileContext(nc) as tc, ExitStack() as ctx:
        if loop_n > 1:
            ctx.enter_context(tc.For_i(0, loop_n, 1))
        const = ctx.enter_context(tc.tile_pool(name="const", bufs=1))
        work = ctx.enter_context(tc.tile_pool(name="work", bufs=1))

        # ---- constants / persistent SBUF tensors (batched DMAs) ----
        w_sb = const.tile([128, 3 * 128], bf16, tag="w_sb")
        nc.sync.dma_start(out=w_sb, in_=wpack[:, :])
        wq, wk, wv = (w_sb[:, 128 * i : 128 * (i + 1)] for i in range(3))
        xq_sb = const.tile([128, QCHUNK], bf16, tag="xq_sb")
        nc.sync.dma_start(out=xq_sb, in_=xqT[:, :])
        wo_sb = const.tile([128, 2, 128], bf16, tag="wo_sb")
        src = bass.AP(tensor=woT, offset=0, ap=[[128, 128], [128 * 128, 2], [1, 128]])
        nc.sync.dma_start(out=wo_sb, in_=src)
        c_sb = const.tile([1, 2 * D], bf16, tag="c_sb")
        nc.sync.dma_start(out=c_sb, in_=cpack[:, :])
        xk_sb = const.tile([128, S], bf16, tag="xk_sb")
        for half in range(2):
            sl = slice(half * 2048, (half + 1) * 2048)
            nc.gpsimd.dma_start(out=xk_sb[:, sl], in_=xkT[:, sl])

        ones1 = const.tile([1, 512], bf16, tag="ones1")
        nc.vector.memset(ones1, 1.0)
        zrow = const.tile([1, 512], bf16, tag="zrow")
        nc.vector.memset(zrow, 0.0)

        kT_sb = const.tile([128, S], bf16, tag="kT_sb")
        qT_sb = const.tile([128, QCHUNK], bf16, tag="qT_sb")
        # v_sb[key, kb, head, 0:32]=v, [...,32]=1.0, [...,33:64]=junk
        # (PSUM rows 33:64/97:128 that the junk feeds are never read)
        v_sb = const.tile([128, NKB, H, 64], bf16, tag="v_sb")
        nc.vector.memset(v_sb[:, :, :, 32], 1.0)

        # ---- prologue: q projections + key chunk 0 (chunks 1-7 are
        # interleaved into the qh0 rounds via st-ring PSUM tiles) ----
        def chunk_mms(pk_ap, pv_ap, c8):
            sl = slice(c8 * 512, (c8 + 1) * 512)
            nc.tensor.matmul(pk_ap, wk, xk_sb[:, sl], start=True, stop=True)
            for j in range(4):
                kb = c8 * 4 + j
                ksl = slice(kb * 128, (kb + 1) * 128)
                nc.tensor.matmul(pv_ap[:, j * 128 : (j + 1) * 128],
                                 xk_sb[:, ksl], wv, start=True, stop=True)

        def chunk_copies(pk_ap, pv_ap, c8):
            sl = slice(c8 * 512, (c8 + 1) * 512)
            kslb = slice(c8 * 4, (c8 + 1) * 4)
            pv4 = pv_ap.rearrange("p (k i h) -> p k i h", k=4, i=H)
            if c8 % 2 == 0:
                nc.scalar.copy(out=kT_sb[:, sl], in_=pk_ap)
                nc.vector.tensor_copy(out=v_sb[:, kslb, :, 0:32], in_=pv4)
            else:
                nc.vector.tensor_copy(out=kT_sb[:, sl], in_=pk_ap)
                nc.scalar.copy(out=v_sb[:, kslb, :, 0:32], in_=pv4)

        with tc.tile_pool(name="proj_ps", bufs=1, space="PSUM") as proj_ps:
            for qh in range(2):
                sl = slice(qh * 512, (qh + 1) * 512)
                pq = proj_ps.tile([128, 512], f32, tag="pk", bufs=2, name="pq")
                nc.tensor.matmul(pq, wq, xq_sb[:, sl], start=True, stop=True)
                nc.vector.tensor_copy(out=qT_sb[:, sl], in_=pq)
            p0 = proj_ps.tile([128, 1024], f32, tag="p0", name="p0")
            chunk_mms(p0[:, 0:512], p0[:, 512:1024], 0)
            chunk_copies(p0[:, 0:512], p0[:, 512:1024], 0)

        # ---- main rounds: scores^T -> exp -> z^T accumulation ----
        round_ctx = ExitStack()
        zden_ps = round_ctx.enter_context(
            tc.tile_pool(name="zden_ps", bufs=1, space="PSUM"))
        st_ps = round_ctx.enter_context(
            tc.tile_pool(name="st_ps", bufs=3, space="PSUM"))
        exp_pool = round_ctx.enter_context(tc.tile_pool(name="exp_pool", bufs=2))

        dram_pool = ctx.enter_context(
            tc.tile_pool(name="dram_pool", bufs=1, space="DRAM")
        )
        den_dram = [
            dram_pool.tile([4, 512], f32, tag=f"dd_{qh}", name=f"dd_{qh}")
            for qh in range(2)
        ]
        rec_dram = [
            dram_pool.tile([128, 16], f32, tag=f"rd_{qh}", name=f"rd_{qh}")
            for qh in range(2)
        ]
        den_sb = work.tile([128, 1024], f32, tag="den_sb")
        rec16 = [work.tile([128, 16], f32, tag=f"rec16_{qh}", name=f"rec16_{qh}")
                 for qh in range(2)]
        rep = [work.tile([128, 512], f32, tag=f"rep_{p}", name=f"rep_{p}")
               for p in range(2)]
        zT_sb = [work.tile([128, QCHUNK], bf16, tag=f"zT_{p}", name=f"zT_{p}")
                 for p in range(2)]

        # z/denominator accumulators: [pair] -> [128, 512] for the current
        # q-half; rows 0:32 z of head 2p, row 32 its denom, rows 64:96 z of
        # head 2p+1, row 96 its denom. qh1 reuses qh0's banks (bufs=1 tags)
        # once qh0's normalization has read them.
        z_cur = [None, None]

        def start_qh():
            for p in range(2):
                z_cur[p] = zden_ps.tile(
                    [128, 512], f32, tag=f"z_{p}", name=f"z_{p}"
                )
                nc.tensor.matmul(
                    z_cur[p], zrow[:, 0:128], zrow, start=True, stop=False,
                    skip_group_check=True,
                )

        def emit_scores(qh, kb):
            qsl = slice(qh * 512, (qh + 1) * 512)
            ksl = slice(kb * 128, (kb + 1) * 128)
            sts = []
            for p in range(2):
                st = st_ps.tile([128, 1024], f32, tag="st", name=f"st_{p}")
                for j in range(2):
                    i = 2 * p + j
                    nc.tensor.matmul(
                        st[:, j * 512 : (j + 1) * 512],
                        kT_sb[32 * i : 32 * (i + 1), ksl],
                        qT_sb[32 * i : 32 * (i + 1), qsl],
                        start=True,
                        stop=True,
                        tile_position=(32 * i, 0),
                    )
                sts.append(st)
            return sts

        def emit_exp(sts, engines):
            ex = [None, None]
            for p in range(2):
                st = sts[p]
                e = exp_pool.tile([128, 1024], bf16, tag=f"ex_{p}", name=f"ex_{p}")
                if engines[p] == "S":
                    nc.scalar.activation(
                        out=e, in_=bf16_hi_alias(st[:, :]), func=AF.Exp,
                        scale=ACT_SCALE,
                    )
                else:
                    src = bf16_hi_alias(st[:, :]) if ve_hi else st[:, :]
                    nc.vector._custom_dve(
                        exp2_op, out=i16_alias(e[:, :]), in0=src,
                        s0=1.0 / 256.0, s1=(127.0 - EXP2_SIGMA) * 128.0,
                    )
                ex[p] = e
            return ex

        def emit_z(kb, ex):
            for p in range(2):
                for j in range(2):
                    nc.tensor.matmul(
                        z_cur[p][64 * j : 64 * j + 64, :],
                        v_sb[:, kb, 2 * p + j, :],
                        ex[p][:, j * 512 : (j + 1) * 512],
                        start=False,
                        stop=(kb == NKB - 1),
                        tile_position=(0, 64 * j),
                        skip_group_check=True,
                    )

        def emit_epilogue(qh):
            # per-qh normalization, overlapped with the next qh's rounds:
            # denom rows (PSUM partitions 32/96) -> SBUF -> DRAM -> reshaped
            # [128,16] for a cheap wide reciprocal -> DRAM -> partition-
            # broadcast back over the z rows.
            qsl = slice(qh * 512, (qh + 1) * 512)
            for p in range(2):
                for j in range(2):
                    r = 64 * j + 32
                    csl = slice(p * 512, (p + 1) * 512)
                    if p == 0:
                        nc.scalar.copy(out=den_sb[r : r + 1, csl],
                                       in_=z_cur[p][r : r + 1, :])
                    else:
                        nc.vector.tensor_copy(out=den_sb[r : r + 1, csl],
                                              in_=z_cur[p][r : r + 1, :])
            # den_dram rows: h = 2*p + j  <- den_sb row 32+64j, cols p*512
            dd = den_dram[qh]
            for j in range(2):
                r = 64 * j + 32
                dst = bass.AP(tensor=dd.tensor, offset=dd.offset + j * 512,
                              ap=[[1024, 2], [1, 512]])  # (p, q)
                nc.sync.dma_start(out=dst, in_=den_sb[r : r + 1, 0:1024])
            # gather all 4 heads' denoms as [128, 16]
            gsrc = bass.AP(tensor=dd.tensor, offset=dd.offset,
                           ap=[[16, 128], [1, 16]])
            nc.sync.dma_start(out=rec16[qh], in_=gsrc)
            nc.vector.reciprocal(out=rec16[qh], in_=rec16[qh])
            nc.sync.dma_start(out=rec_dram[qh], in_=rec16[qh])
            for p in range(2):
                # rep[p] rows 0:32 <- head 2p, rows 64:96 <- head 2p+1
                for j in range(2):
                    h = 2 * p + j
                    srcap = bass.AP(tensor=rec_dram[qh].tensor,
                                    offset=rec_dram[qh].offset + h * 512,
                                    ap=[[0, 32], [1, 512]])
                    nc.sync.dma_start(out=rep[p][64 * j : 64 * j + 32, :],
                                      in_=srcap)
                    rsl = slice(64 * j, 64 * j + 32)
                    nc.vector.tensor_mul(
                        zT_sb[p][rsl, qsl], z_cur[p][rsl, :], rep[p][rsl, :]
                    )

        # engine assignment: p0 -> ScalarE, p1 -> VectorE; optionally every
        # flip_every-th round sends p1 to ScalarE too (rebalance knob).
        def engines_for(k):
            if flip_every and k % flip_every == flip_every - 1:
                return ("S", "S")
            return ("S", "V")

        for qh in range(2):
            start_qh()
            sts = emit_scores(qh, 0)
            for kb in range(NKB):
                ex = emit_exp(sts, engines_for(qh * NKB + kb))
                if qh == 0 and kb % 4 == 1 and kb // 4 < 7:
                    c8 = kb // 4 + 1
                    ct = st_ps.tile([128, 1024], f32, tag="st", name=f"ck_{c8}")
                    chunk_mms(ct[:, 0:512], ct[:, 512:1024], c8)
                    chunk_copies(ct[:, 0:512], ct[:, 512:1024], c8)
                if kb + 1 < NKB:
                    sts = emit_scores(qh, kb + 1)
                emit_z(kb, ex)
            emit_epilogue(qh)

        round_ctx.close()

        # ---- final projection: out^T[d, q] per q-half (W_O stationary,
        # z^T moving at N=512); blend constant via two K=1 bf16 matmuls ----
        with tc.tile_pool(name="u_ps", bufs=2, space="PSUM") as u_ps, tc.tile_pool(
            name="out_pool", bufs=2
        ) as out_pool:
            for qh in range(2):
                qsl = slice(qh * 512, (qh + 1) * 512)
                ue = u_ps.tile([128, 512], f32, tag="ue")  # heads at rows 0:32
                uo = u_ps.tile([128, 512], f32, tag="uo")  # heads at rows 64:96
                nc.tensor.matmul(
                    ue, wo_sb[0:32, 0, :], zT_sb[0][0:32, qsl], start=True,
                    stop=False, skip_group_check=True, tile_position=(0, 0),
                )
                nc.tensor.matmul(
                    ue, wo_sb[0:32, 1, :], zT_sb[1][0:32, qsl], start=False,
                    stop=False, skip_group_check=True, tile_position=(0, 0),
                )
                nc.tensor.matmul(
                    ue, c_sb[0:1, 0:128], ones1, start=False, stop=False,
                    skip_group_check=True, tile_position=(0, 0),
                )
                nc.tensor.matmul(
                    ue, c_sb[0:1, 128:256], ones1, start=False, stop=True,
                    skip_group_check=True, tile_position=(0, 0),
                )
                nc.tensor.matmul(
                    uo, wo_sb[64:96, 0, :], zT_sb[0][64:96, qsl], start=True,
                    stop=False, skip_group_check=True, tile_position=(64, 0),
                )
                nc.tensor.matmul(
                    uo, wo_sb[64:96, 1, :], zT_sb[1][64:96, qsl], start=False,
                    stop=True, skip_group_check=True, tile_position=(64, 0),
                )
                ob = out_pool.tile([128, 512], f32, tag="ob")
                nc.scalar.copy(out=ob, in_=ue)
                nc.vector.tensor_add(ob, ob, uo)
                q = nc.sync if qh == 0 else nc.gpsimd
                q.dma_start(out=out[:, qsl], in_=ob)

    nc.compile()
    return nc


def _get_program(loop_n: int = 1):
    import os

    global _PROGRAM
    ve_hi = os.environ.get("BASS_VE_HI", "1") == "1"
    flip = int(os.environ.get("BASS_FLIP", "10"))
    if loop_n != 1:
        return _build_program(loop_n, ve_hi, flip)
    if _PROGRAM is None:
        _PROGRAM = _build_program(1, ve_hi, flip)
    return _PROGRAM


def make_in_maps(x, W_K, W_Q, W_V, W_O):
    x = np.asarray(x, np.float32)
    W_K = np.asarray(W_K, np.float32)
    W_Q = np.asarray(W_Q, np.float32)
    W_V = np.asarray(W_V, np.float32)
    W_O = np.asarray(W_O, np.float32)

    wqT = np.ascontiguousarray((W_Q.transpose(2, 0, 1).reshape(D, H * DH)) * PRESCALE)
    wkT = np.ascontiguousarray(W_K.transpose(2, 0, 1).reshape(D, H * DH))
    wvT = np.ascontiguousarray(W_V.transpose(2, 0, 1).reshape(D, H * DH))
    wpack = np.concatenate([wqT, wkT, wvT], axis=1).astype(BF16)
    woT_flat = 0.5 * W_O.T  # [f, d']
    woT = np.zeros((2, 128, D), np.float32)
    for p in range(2):
        woT[p, 0:32] = woT_flat[(2 * p) * 32 : (2 * p) * 32 + 32]
        woT[p, 64:96] = woT_flat[(2 * p + 1) * 32 : (2 * p + 1) * 32 + 32]

    in_maps = []
    for core in range(NCORES):
        b, qc = divmod(core, 4)
        xb = x[b]
        xkT_b = np.ascontiguousarray(xb.T).astype(BF16)
        xqT_c = np.ascontiguousarray(xb[qc * QCHUNK : (qc + 1) * QCHUNK].T).astype(BF16)
        # exact blend constant: c = 0.5 * (sum_k v[k]) @ W_O^T, split into
        # bf16 hi + lo for two exact-ish K=1 matmuls
        sv = (xb.sum(0, dtype=np.float64) @ wvT.astype(np.float64))
        c = (0.5 * (sv @ W_O.T.astype(np.float64))).astype(np.float32)
        c_hi = c.astype(BF16)
        c_lo = (c - c_hi.astype(np.float32)).astype(BF16)
        cpack = np.concatenate([c_hi, c_lo])[None, :].astype(BF16)
        in_maps.append(
            {
                "xkT": xkT_b,
                "xqT": xqT_c,
                "wpack": wpack,
                "woT": woT.astype(BF16),
                "cpack": cpack,
            }
        )
    return in_maps


def kernel(x, W_K, W_Q, W_V, W_O):
    from concourse.bass_utils import run_bass_kernel_spmd

    nc = _get_program()
    in_maps = make_in_maps(x, W_K, W_Q, W_V, W_O)
    res = run_bass_kernel_spmd(nc, in_maps, core_ids=list(range(NCORES)))
    full = np.empty((B, S, D), np.float32)
    for core in range(NCORES):
        b, qc = divmod(core, 4)
        full[b, qc * QCHUNK : (qc + 1) * QCHUNK, :] = res.results[core]["out"].T
    return full


# revision 15
# speedup vs baseline: 1.0448x; 1.0448x over previous
"""Trainium2 Bass kernel for a 4-head attention layer with post-softmax
affine blend (attn = 0.5*softmax(qk/sqrt(dh)) + 0.5), distributed over 8
NeuronCores.

Reference computation (B=2, S=4096, D=128, H=4, Dh=32):
    k = einsum('ihd,bpd->biph', W_K, x)
    q = einsum('ihd,bpd->biph', W_Q, x)
    v = einsum('ihd,bpd->biph', W_V, x)
    scores = einsum('biph,biqh->biqp', k, q) / sqrt(32)
    attn   = softmax(scores, -1) * 0.5 + 0.5
    z      = einsum('biph,biqp->biqh', v, attn)
    out    = einsum('df,bpf->bpd', W_O, z_flat)

Sharding: 8 cores = (batch b in {0,1}) x (query chunk qc in 4 x 1024).
Each core computes all 4 heads for its 1024 queries against all 4096
keys and emits the disjoint output slice out[b, qc*1024:(qc+1)*1024, :]
(transposed on-chip as [D, 1024]; the host unshard transposes back).

Per-core algorithm (everything stays on-chip):
  - Prologue: batched input DMAs split over two queues, then a dense
    projection burst (k^T/q^T/v on TensorE) that also un-throttles the
    PE HAM clock gate.
  - Main rounds (2 q-halves x 32 key-blocks): scores^T tiles
    [128 keys x 2x512 q] via 4 row-packed K=32 matmuls into a 3-deep
    PSUM ring; exp is computed from PSUM split across ScalarE
    (activation, scale folded into W_Q) and VectorE (Schraudolph exp2
    custom DVE op writing bf16 bits via int16); attn@v accumulates
    z^T in PSUM with col-packed M=64 matmuls whose stationary operand
    [v_i | ones | 0] also accumulates the softmax denominator.
  - Epilogue per q-half: denominators are reshaped through DRAM onto
    128 partitions for a cheap VectorE reciprocal, broadcast back, and
    applied to z^T.
  - Final projection is transposed (W_O slices stationary, z^T moving,
    N=512): out^T accumulates in PSUM; the uniform 0.5*sum_k(v) blend
    constant is added via two K=1 bf16 hi/lo matmuls.
"""

import math

import numpy as np
import ml_dtypes

BF16 = ml_dtypes.bfloat16

B, S, D, H, DH = 2, 4096, 128, 4, 32
QCHUNK = 1024  # queries per core
NCORES = 8
NKB = S // 128  # 32 key blocks
# exp(s) is computed as exp((s * 2^15 * log2(e)) * ln(2) / 2^15); the big
# pre-scale is folded into W_Q so a bit-trick exp2 on VectorE can share the
# same score tensor.
PRESCALE = (2.0**15) * math.log2(math.e) / math.sqrt(DH)
ACT_SCALE = math.log(2.0) / (2.0**15)

# Schraudolph exp2 constant: sigma balances the multiplicative error of the
# linear-mantissa approximation; folded into the int16 bf16-bit construction.
EXP2_SIGMA = 0.02979

_PROGRAM = None


def _register_exp2():
    """Register (once) a fused y = x*C0 + C1 custom DVE op whose int16
    output, reinterpreted as bf16, is 2^(x/2^15) a la Schraudolph."""
    from concourse import dve_ops
    from concourse.dve_spec import Spec, Src0, C0, C1, lower, _has_src1
    from concourse.dve_uop import DveOpSpec

    name = "EXP2_SCHRAU_ANT"
    for o in dve_ops.OPS:
        if o.name == name:
            return o
    spec = Spec(body=Src0 * C0 + C1,
                reference=lambda in0, in1, c0, c1, c2: in0 * c0 + c1)
    opcode = dve_ops._CUSTOM_DVE_ROW_BASE + len(dve_ops.OPS)
    shas = {}
    for ver in ("v3", "v4"):
        s = DveOpSpec(name=name, opcode=opcode, uops=lower(spec, ver=ver),
                      rd1_en=_has_src1(spec))
        shas[ver] = s.sha(ver)
    op = dve_ops.DveOp(name, spec, subdim=False, uops_sha=shas)
    dve_ops.OPS.append(op)
    dve_ops.CUSTOM_DVE_SPECS[name] = spec
    dve_ops._SUB_OPCODE_FOR_NAME[name] = opcode
    return op


def _build_program(loop_n: int = 1, ve_hi: bool = True, flip_every: int = 10):
    import concourse.bass as bass
    import concourse.mybir as mybir
    import concourse.tile as tile
    from concourse import bacc
    from contextlib import ExitStack

    import dataclasses

    f32 = mybir.dt.float32
    bf16 = mybir.dt.bfloat16
    AF = mybir.ActivationFunctionType
    exp2_op = _register_exp2()

    def i16_alias(ap):
        h = dataclasses.replace(ap.tensor, dtype=mybir.dt.int16)
        return bass.AP(tensor=h, offset=ap.offset, ap=[list(d) for d in ap.ap])

    def bf16_hi_alias(ap):
        """View an f32 [P, N] AP as the bf16 high halves: [P, N] bf16,
        element stride 2, offset +1 (little-endian high 2 bytes)."""
        h = dataclasses.replace(
            ap.tensor, dtype=mybir.dt.bfloat16,
            shape=[ap.tensor.shape[0], ap.tensor.shape[1] * 2],
        )
        newap = [[ap.ap[0][0] * 2, ap.ap[0][1]]] + [
            [d[0] * 2, d[1]] for d in ap.ap[1:]
        ]
        return bass.AP(tensor=h, offset=ap.offset * 2 + 1, ap=newap)

    def pstride(tile_ap, row0, step, n, col0, ncols):
        """Partition-strided view of a [128, C] tile: rows row0, row0+step,
        ... (n of them), cols col0:col0+ncols."""
        rs = tile_ap.ap[0][0]
        cs = tile_ap.ap[1][0]
        return bass.AP(
            tensor=tile_ap.tensor,
            offset=tile_ap.offset + row0 * rs + col0 * cs,
            ap=[[rs * step, n], [cs, ncols]],
        )

    nc = bacc.Bacc(None, target_bir_lowering=False)

    xkT = nc.dram_tensor("xkT", [D, S], bf16, kind="ExternalInput")
    xqT = nc.dram_tensor("xqT", [D, QCHUNK], bf16, kind="ExternalInput")
    # wpack: cols 0:128 wq (pre-scaled), 128:256 wk, 256:384 wv
    wpack = nc.dram_tensor("wpack", [D, 3 * 128], bf16, kind="ExternalInput")
    # woT[p, r, :]: rows 0:32 head 2p, rows 64:96 head 2p+1 (0.5*W_O.T slices)
    woT = nc.dram_tensor("woT", [2, 128, D], bf16, kind="ExternalInput")
    # cpack: cols 0:128 bf16 hi of blend constant c, 128:256 lo residual
    cpack = nc.dram_tensor("cpack", [1, 2 * D], bf16, kind="ExternalInput")
    out = nc.dram_tensor("out", [D, QCHUNK], f32, kind="ExternalOutput")

    with tile.TileContext(nc) as tc, ExitStack() as ctx:
        if loop_n > 1:
            ctx.enter_context(tc.For_i(0, loop_n, 1))
        const = ctx.enter_context(tc.tile_pool(name="const", bufs=1))
        work = ctx.enter_context(tc.tile_pool(name="work", bufs=1))

        # ---- constants / persistent SBUF tensors (batched DMAs) ----
        w_sb = const.tile([128, 3 * 128], bf16, tag="w_sb")
        nc.sync.dma_start(out=w_sb, in_=wpack[:, :])
        wq, wk, wv = (w_sb[:, 128 * i : 128 * (i + 1)] for i in range(3))
        xq_sb = const.tile([128, QCHUNK], bf16, tag="xq_sb")
        nc.sync.dma_start(out=xq_sb, in_=xqT[:, :])
        wo_sb = const.tile([128, 2, 128], bf16, tag="wo_sb")
        src = bass.AP(tensor=woT, offset=0, ap=[[128, 128], [128 * 128, 2], [1, 128]])
        nc.sync.dma_start(out=wo_sb, in_=src)
        c_sb = const.tile([1, 2 * D], bf16, tag="c_sb")
        nc.sync.dma_start(out=c_sb, in_=cpack[:, :])
        xk_sb = const.tile([128, S], bf16, tag="xk_sb")
        for half in range(2):
            sl = slice(half * 2048, (half + 1) * 2048)
            nc.gpsimd.dma_start(out=xk_sb[:, sl], in_=xkT[:, sl])

        ones1 = const.tile([1, 512], bf16, tag="ones1")
        nc.vector.memset(ones1, 1.0)
        zrow = const.tile([1, 512], bf16, tag="zrow")
        nc.vector.memset(zrow, 0.0)

        kT_sb = const.tile([128, S], bf16, tag="kT_sb")
        qT_sb = const.tile([128, QCHUNK], bf16, tag="qT_sb")
        # v_sb[key, kb, head, 0:32]=v, [...,32]=1.0, [...,33:64]=junk
        # (PSUM rows 33:64/97:128 that the junk feeds are never read)
        v_sb = const.tile([128, NKB, H, 64], bf16, tag="v_sb")
        nc.vector.memset(v_sb[:, :, :, 32], 1.0)

        # ---- prologue: q projections + key chunk 0 (chunks 1-7 are
        # interleaved into the qh0 rounds via st-ring PSUM tiles) ----
        def chunk_mms(pk_ap, pv_ap, c8):
            sl = slice(c8 * 512, (c8 + 1) * 512)
            nc.tensor.matmul(pk_ap, wk, xk_sb[:, sl], start=True, stop=True)
            for j in range(4):
                kb = c8 * 4 + j
                ksl = slice(kb * 128, (kb + 1) * 128)
                nc.tensor.matmul(pv_ap[:, j * 128 : (j + 1) * 128],
                                 xk_sb[:, ksl], wv, start=True, stop=True)

        def chunk_copies(pk_ap, pv_ap, c8):
            sl = slice(c8 * 512, (c8 + 1) * 512)
            kslb = slice(c8 * 4, (c8 + 1) * 4)
            pv4 = pv_ap.rearrange("p (k i h) -> p k i h", k=4, i=H)
            if c8 % 2 == 0:
                nc.scalar.copy(out=kT_sb[:, sl], in_=pk_ap)
                nc.vector.tensor_copy(out=v_sb[:, kslb, :, 0:32], in_=pv4)
            else:
                nc.vector.tensor_copy(out=kT_sb[:, sl], in_=pk_ap)
                nc.scalar.copy(out=v_sb[:, kslb, :, 0:32], in_=pv4)

        with tc.tile_pool(name="proj_ps", bufs=1, space="PSUM") as proj_ps:
            for qh in range(2):
                sl = slice(qh * 512, (qh + 1) * 512)
                pq = proj_ps.tile([128, 512], f32, tag="pk", bufs=2, name="pq")
                nc.tensor.matmul(pq, wq, xq_sb[:, sl], start=True, stop=True)
                nc.vector.tensor_copy(out=qT_sb[:, sl], in_=pq)
            p0 = proj_ps.tile([128, 1024], f32, tag="p0", name="p0")
            chunk_mms(p0[:, 0:512], p0[:, 512:1024], 0)
            chunk_copies(p0[:, 0:512], p0[:, 512:1024], 0)

        # ---- main rounds: scores^T -> exp -> z^T accumulation ----
        round_ctx = ExitStack()
        zden_ps = round_ctx.enter_context(
            tc.tile_pool(name="zden_ps", bufs=1, space="PSUM"))
        st_ps = round_ctx.enter_context(
            tc.tile_pool(name="st_ps", bufs=3, space="PSUM"))
        exp_pool = round_ctx.enter_context(tc.tile_pool(name="exp_pool", bufs=2))

        dram_pool = ctx.enter_context(
            tc.tile_pool(name="dram_pool", bufs=1, space="DRAM")
        )
        den_dram = [
            dram_pool.tile([4, 512], f32, tag=f"dd_{qh}", name=f"dd_{qh}")
            for qh in range(2)
        ]
        rec_dram = [
            dram_pool.tile([128, 16], f32, tag=f"rd_{qh}", name=f"rd_{qh}")
            for qh in range(2)
        ]
        den_sb = work.tile([128, 1024], f32, tag="den_sb")
        zc_sb = [work.tile([128, 512], f32, tag=f"zc_{p}", name=f"zc_{p}")
                 for p in range(2)]
        rec16 = [work.tile([128, 16], f32, tag=f"rec16_{qh}", name=f"rec16_{qh}")
                 for qh in range(2)]
        rep = [work.tile([128, 512], f32, tag=f"rep_{p}", name=f"rep_{p}")
               for p in range(2)]
        zT_sb = [work.tile([128, QCHUNK], bf16, tag=f"zT_{p}", name=f"zT_{p}")
                 for p in range(2)]

        # z/denominator accumulators: [pair] -> [128, 512] for the current
        # q-half; rows 0:32 z of head 2p, row 32 its denom, rows 64:96 z of
        # head 2p+1, row 96 its denom. qh1 reuses qh0's banks (bufs=1 tags)
        # once qh0's normalization has read them.
        z_cur = [None, None]

        def start_qh():
            for p in range(2):
                z_cur[p] = zden_ps.tile(
                    [128, 512], f32, tag=f"z_{p}", name=f"z_{p}"
                )
                nc.tensor.matmul(
                    z_cur[p], zrow[:, 0:128], zrow, start=True, stop=False,
                    skip_group_check=True,
                )

        def emit_scores(qh, kb):
            qsl = slice(qh * 512, (qh + 1) * 512)
            ksl = slice(kb * 128, (kb + 1) * 128)
            sts = []
            for p in range(2):
                st = st_ps.tile([128, 1024], f32, tag="st", name=f"st_{p}")
                for j in range(2):
                    i = 2 * p + j
                    nc.tensor.matmul(
                        st[:, j * 512 : (j + 1) * 512],
                        kT_sb[32 * i : 32 * (i + 1), ksl],
                        qT_sb[32 * i : 32 * (i + 1), qsl],
                        start=True,
                        stop=True,
                        tile_position=(32 * i, 0),
                    )
                sts.append(st)
            return sts

        def emit_exp(sts, engines):
            ex = [None, None]
            for p in range(2):
                st = sts[p]
                e = exp_pool.tile([128, 1024], bf16, tag=f"ex_{p}", name=f"ex_{p}")
                if engines[p] == "S":
                    nc.scalar.activation(
                        out=e, in_=bf16_hi_alias(st[:, :]), func=AF.Exp,
                        scale=ACT_SCALE,
                    )
                else:
                    src = bf16_hi_alias(st[:, :]) if ve_hi else st[:, :]
                    nc.vector._custom_dve(
                        exp2_op, out=i16_alias(e[:, :]), in0=src,
                        s0=1.0 / 256.0, s1=(127.0 - EXP2_SIGMA) * 128.0,
                    )
                ex[p] = e
            return ex

        def emit_z(kb, ex):
            for p in range(2):
                for j in range(2):
                    nc.tensor.matmul(
                        z_cur[p][64 * j : 64 * j + 64, :],
                        v_sb[:, kb, 2 * p + j, :],
                        ex[p][:, j * 512 : (j + 1) * 512],
                        start=False,
                        stop=(kb == NKB - 1),
                        tile_position=(0, 64 * j),
                        skip_group_check=True,
                    )

        def emit_epilogue(qh):
            # per-qh normalization, overlapped with the next qh's rounds:
            # denom rows (PSUM partitions 32/96) -> SBUF -> DRAM -> reshaped
            # [128,16] for a cheap wide reciprocal -> DRAM -> partition-
            # broadcast back over the z rows.
            qsl = slice(qh * 512, (qh + 1) * 512)
            # free the z PSUM banks ASAP (the next q-half's zeroing matmul
            # WAR-waits on all reads): pull z rows + denom rows into SBUF
            # with one [128,512] copy per pair, then normalize from SBUF.
            for p in range(2):
                if p == 0:
                    nc.scalar.copy(out=zc_sb[p], in_=z_cur[p])
                else:
                    nc.vector.tensor_copy(out=zc_sb[p], in_=z_cur[p])
            for p in range(2):
                for j in range(2):
                    r = 64 * j + 32
                    csl = slice(p * 512, (p + 1) * 512)
                    if p == 0:
                        nc.scalar.copy(out=den_sb[r : r + 1, csl],
                                       in_=zc_sb[p][r : r + 1, :])
                    else:
                        nc.vector.tensor_copy(out=den_sb[r : r + 1, csl],
                                              in_=zc_sb[p][r : r + 1, :])
            # den_dram rows: h = 2*p + j  <- den_sb row 32+64j, cols p*512
            dd = den_dram[qh]
            for j in range(2):
                r = 64 * j + 32
                dst = bass.AP(tensor=dd.tensor, offset=dd.offset + j * 512,
                              ap=[[1024, 2], [1, 512]])  # (p, q)
                nc.sync.dma_start(out=dst, in_=den_sb[r : r + 1, 0:1024])
            # gather all 4 heads' denoms as [128, 16]
            gsrc = bass.AP(tensor=dd.tensor, offset=dd.offset,
                           ap=[[16, 128], [1, 16]])
            nc.sync.dma_start(out=rec16[qh], in_=gsrc)
            nc.vector.reciprocal(out=rec16[qh], in_=rec16[qh])
            nc.sync.dma_start(out=rec_dram[qh], in_=rec16[qh])
            for p in range(2):
                # rep[p] rows 0:32 <- head 2p, rows 64:96 <- head 2p+1
                for j in range(2):
                    h = 2 * p + j
                    srcap = bass.AP(tensor=rec_dram[qh].tensor,
                                    offset=rec_dram[qh].offset + h * 512,
                                    ap=[[0, 32], [1, 512]])
                    nc.sync.dma_start(out=rep[p][64 * j : 64 * j + 32, :],
                                      in_=srcap)
                    rsl = slice(64 * j, 64 * j + 32)
                    nc.vector.tensor_mul(
                        zT_sb[p][rsl, qsl], zc_sb[p][rsl, :], rep[p][rsl, :]
                    )

        # engine assignment: p0 -> ScalarE, p1 -> VectorE; optionally every
        # flip_every-th round sends p1 to ScalarE too (rebalance knob).
        def engines_for(k):
            if flip_every and k % flip_every == flip_every - 1:
                return ("S", "S")
            return ("S", "V")

        for qh in range(2):
            start_qh()
            sts = emit_scores(qh, 0)
            for kb in range(NKB):
                ex = emit_exp(sts, engines_for(qh * NKB + kb))
                if qh == 0 and kb % 4 == 1 and kb // 4 < 7:
                    c8 = kb // 4 + 1
                    ct = st_ps.tile([128, 1024], f32, tag="st", name=f"ck_{c8}")
                    chunk_mms(ct[:, 0:512], ct[:, 512:1024], c8)
                    cur_chunk = ct
                elif qh == 0 and kb % 4 == 2 and kb // 4 < 7:
                    c8 = kb // 4 + 1
                    sl = slice(c8 * 512, (c8 + 1) * 512)
                    if c8 % 2 == 0:
                        nc.scalar.copy(out=kT_sb[:, sl], in_=cur_chunk[:, 0:512])
                    else:
                        nc.vector.tensor_copy(out=kT_sb[:, sl],
                                              in_=cur_chunk[:, 0:512])
                elif qh == 0 and kb % 4 == 3 and kb // 4 < 7:
                    c8 = kb // 4 + 1
                    kslb = slice(c8 * 4, (c8 + 1) * 4)
                    pv4 = cur_chunk[:, 512:1024].rearrange(
                        "p (k i h) -> p k i h", k=4, i=H)
                    if c8 % 2 == 0:
                        nc.vector.tensor_copy(out=v_sb[:, kslb, :, 0:32], in_=pv4)
                    else:
                        nc.scalar.copy(out=v_sb[:, kslb, :, 0:32], in_=pv4)
                if kb + 1 < NKB:
                    sts = emit_scores(qh, kb + 1)
                emit_z(kb, ex)
            emit_epilogue(qh)

        round_ctx.close()

        # ---- final projection: out^T[d, q] per q-half (W_O stationary,
        # z^T moving at N=512); blend constant via two K=1 bf16 matmuls ----
        with tc.tile_pool(name="u_ps", bufs=2, space="PSUM") as u_ps, tc.tile_pool(
            name="out_pool", bufs=2
        ) as out_pool:
            for qh in range(2):
                qsl = slice(qh * 512, (qh + 1) * 512)
                ue = u_ps.tile([128, 512], f32, tag="ue")  # heads at rows 0:32
                uo = u_ps.tile([128, 512], f32, tag="uo")  # heads at rows 64:96
                nc.tensor.matmul(
                    ue, wo_sb[0:32, 0, :], zT_sb[0][0:32, qsl], start=True,
                    stop=False, skip_group_check=True, tile_position=(0, 0),
                )
                nc.tensor.matmul(
                    ue, wo_sb[0:32, 1, :], zT_sb[1][0:32, qsl], start=False,
                    stop=False, skip_group_check=True, tile_position=(0, 0),
                )
                nc.tensor.matmul(
                    ue, c_sb[0:1, 0:128], ones1, start=False, stop=False,
                    skip_group_check=True, tile_position=(0, 0),
                )
                nc.tensor.matmul(
                    ue, c_sb[0:1, 128:256], ones1, start=False, stop=True,
                    skip_group_check=True, tile_position=(0, 0),
                )
                nc.tensor.matmul(
                    uo, wo_sb[64:96, 0, :], zT_sb[0][64:96, qsl], start=True,
                    stop=False, skip_group_check=True, tile_position=(64, 0),
                )
                nc.tensor.matmul(
                    uo, wo_sb[64:96, 1, :], zT_sb[1][64:96, qsl], start=False,
                    stop=True, skip_group_check=True, tile_position=(64, 0),
                )
                ob = out_pool.tile([128, 512], f32, tag="ob")
                nc.scalar.copy(out=ob, in_=ue)
                nc.vector.tensor_add(ob, ob, uo)
                q = nc.sync if qh == 0 else nc.gpsimd
                q.dma_start(out=out[:, qsl], in_=ob)

    nc.compile()
    return nc


def _get_program(loop_n: int = 1):
    import os

    global _PROGRAM
    ve_hi = os.environ.get("BASS_VE_HI", "1") == "1"
    flip = int(os.environ.get("BASS_FLIP", "10"))
    if loop_n != 1:
        return _build_program(loop_n, ve_hi, flip)
    if _PROGRAM is None:
        _PROGRAM = _build_program(1, ve_hi, flip)
    return _PROGRAM


def make_in_maps(x, W_K, W_Q, W_V, W_O):
    x = np.asarray(x, np.float32)
    W_K = np.asarray(W_K, np.float32)
    W_Q = np.asarray(W_Q, np.float32)
    W_V = np.asarray(W_V, np.float32)
    W_O = np.asarray(W_O, np.float32)

    wqT = np.ascontiguousarray((W_Q.transpose(2, 0, 1).reshape(D, H * DH)) * PRESCALE)
    wkT = np.ascontiguousarray(W_K.transpose(2, 0, 1).reshape(D, H * DH))
    wvT = np.ascontiguousarray(W_V.transpose(2, 0, 1).reshape(D, H * DH))
    wpack = np.concatenate([wqT, wkT, wvT], axis=1).astype(BF16)
    woT_flat = 0.5 * W_O.T  # [f, d']
    woT = np.zeros((2, 128, D), np.float32)
    for p in range(2):
        woT[p, 0:32] = woT_flat[(2 * p) * 32 : (2 * p) * 32 + 32]
        woT[p, 64:96] = woT_flat[(2 * p + 1) * 32 : (2 * p + 1) * 32 + 32]

    in_maps = []
    for core in range(NCORES):
        b, qc = divmod(core, 4)
        xb = x[b]
        xkT_b = np.ascontiguousarray(xb.T).astype(BF16)
        xqT_c = np.ascontiguousarray(xb[qc * QCHUNK : (qc + 1) * QCHUNK].T).astype(BF16)
        # exact blend constant: c = 0.5 * (sum_k v[k]) @ W_O^T, split into
        # bf16 hi + lo for two exact-ish K=1 matmuls
        sv = (xb.sum(0, dtype=np.float64) @ wvT.astype(np.float64))
        c = (0.5 * (sv @ W_O.T.astype(np.float64))).astype(np.float32)
        c_hi = c.astype(BF16)
        c_lo = (c - c_hi.astype(np.float32)).astype(BF16)
        cpack = np.concatenate([c_hi, c_lo])[None, :].astype(BF16)
        in_maps.append(
            {
                "xkT": xkT_b,
                "xqT": xqT_c,
                "wpack": wpack,
                "woT": woT.astype(BF16),
                "cpack": cpack,
            }
        )
    return in_maps


def kernel(x, W_K, W_Q, W_V, W_O):
    from concourse.bass_utils import run_bass_kernel_spmd

    nc = _get_program()
    in_maps = make_in_maps(x, W_K, W_Q, W_V, W_O)
    res = run_bass_kernel_spmd(nc, in_maps, core_ids=list(range(NCORES)))
    full = np.empty((B, S, D), np.float32)
    for core in range(NCORES):
        b, qc = divmod(core, 4)
        full[b, qc * QCHUNK : (qc + 1) * QCHUNK, :] = res.results[core]["out"].T
    return full


# revision 18
# speedup vs baseline: 1.0508x; 1.0057x over previous
"""Trainium2 Bass kernel for a 4-head attention layer with post-softmax
affine blend (attn = 0.5*softmax(qk/sqrt(dh)) + 0.5), distributed over 8
NeuronCores.

Reference computation (B=2, S=4096, D=128, H=4, Dh=32):
    k = einsum('ihd,bpd->biph', W_K, x)
    q = einsum('ihd,bpd->biph', W_Q, x)
    v = einsum('ihd,bpd->biph', W_V, x)
    scores = einsum('biph,biqh->biqp', k, q) / sqrt(32)
    attn   = softmax(scores, -1) * 0.5 + 0.5
    z      = einsum('biph,biqp->biqh', v, attn)
    out    = einsum('df,bpf->bpd', W_O, z_flat)

Sharding: 8 cores = (batch b in {0,1}) x (query chunk qc in 4 x 1024).
Each core computes all 4 heads for its 1024 queries against all 4096
keys and emits the disjoint output slice out[b, qc*1024:(qc+1)*1024, :]
(transposed on-chip as [D, 1024]; the host unshard transposes back).

Per-core algorithm (everything stays on-chip):
  - Prologue: batched input DMAs split over two queues, then a dense
    projection burst (k^T/q^T/v on TensorE) that also un-throttles the
    PE HAM clock gate.
  - Main rounds (2 q-halves x 32 key-blocks): scores^T tiles
    [128 keys x 2x512 q] via 4 row-packed K=32 matmuls into a 3-deep
    PSUM ring; exp is computed from PSUM split across ScalarE
    (activation, scale folded into W_Q) and VectorE (Schraudolph exp2
    custom DVE op writing bf16 bits via int16); attn@v accumulates
    z^T in PSUM with col-packed M=64 matmuls whose stationary operand
    [v_i | ones | 0] also accumulates the softmax denominator.
  - Epilogue per q-half: denominators are reshaped through DRAM onto
    128 partitions for a cheap VectorE reciprocal, broadcast back, and
    applied to z^T.
  - Final projection is transposed (W_O slices stationary, z^T moving,
    N=512): out^T accumulates in PSUM; the uniform 0.5*sum_k(v) blend
    constant is added via two K=1 bf16 hi/lo matmuls.
"""

import math

import numpy as np
import ml_dtypes

BF16 = ml_dtypes.bfloat16

B, S, D, H, DH = 2, 4096, 128, 4, 32
QCHUNK = 1024  # queries per core
NCORES = 8
NKB = S // 128  # 32 key blocks
# exp(s) is computed as exp((s * 2^15 * log2(e)) * ln(2) / 2^15); the big
# pre-scale is folded into W_Q so a bit-trick exp2 on VectorE can share the
# same score tensor.
PRESCALE = (2.0**15) * math.log2(math.e) / math.sqrt(DH)
ACT_SCALE = math.log(2.0) / (2.0**15)

# Schraudolph exp2 constant: sigma balances the multiplicative error of the
# linear-mantissa approximation; folded into the int16 bf16-bit construction.
EXP2_SIGMA = 0.02979

_PROGRAM = None


def _register_exp2():
    """Register (once) a fused y = x*C0 + C1 custom DVE op whose int16
    output, reinterpreted as bf16, is 2^(x/2^15) a la Schraudolph."""
    from concourse import dve_ops
    from concourse.dve_spec import Spec, Src0, C0, C1, lower, _has_src1
    from concourse.dve_uop import DveOpSpec

    name = "EXP2_SCHRAU_ANT"
    for o in dve_ops.OPS:
        if o.name == name:
            return o
    spec = Spec(body=Src0 * C0 + C1,
                reference=lambda in0, in1, c0, c1, c2: in0 * c0 + c1)
    opcode = dve_ops._CUSTOM_DVE_ROW_BASE + len(dve_ops.OPS)
    shas = {}
    for ver in ("v3", "v4"):
        s = DveOpSpec(name=name, opcode=opcode, uops=lower(spec, ver=ver),
                      rd1_en=_has_src1(spec))
        shas[ver] = s.sha(ver)
    op = dve_ops.DveOp(name, spec, subdim=False, uops_sha=shas)
    dve_ops.OPS.append(op)
    dve_ops.CUSTOM_DVE_SPECS[name] = spec
    dve_ops._SUB_OPCODE_FOR_NAME[name] = opcode
    return op


def _build_program(loop_n: int = 1, ve_hi: bool = True, flip_every: int = 10):
    import concourse.bass as bass
    import concourse.mybir as mybir
    import concourse.tile as tile
    from concourse import bacc
    from contextlib import ExitStack

    import dataclasses

    f32 = mybir.dt.float32
    bf16 = mybir.dt.bfloat16
    AF = mybir.ActivationFunctionType
    exp2_op = _register_exp2()

    def i16_alias(ap):
        h = dataclasses.replace(ap.tensor, dtype=mybir.dt.int16)
        return bass.AP(tensor=h, offset=ap.offset, ap=[list(d) for d in ap.ap])

    def bf16_hi_alias(ap):
        """View an f32 [P, N] AP as the bf16 high halves: [P, N] bf16,
        element stride 2, offset +1 (little-endian high 2 bytes)."""
        h = dataclasses.replace(
            ap.tensor, dtype=mybir.dt.bfloat16,
            shape=[ap.tensor.shape[0], ap.tensor.shape[1] * 2],
        )
        newap = [[ap.ap[0][0] * 2, ap.ap[0][1]]] + [
            [d[0] * 2, d[1]] for d in ap.ap[1:]
        ]
        return bass.AP(tensor=h, offset=ap.offset * 2 + 1, ap=newap)

    def pstride(tile_ap, row0, step, n, col0, ncols):
        """Partition-strided view of a [128, C] tile: rows row0, row0+step,
        ... (n of them), cols col0:col0+ncols."""
        rs = tile_ap.ap[0][0]
        cs = tile_ap.ap[1][0]
        return bass.AP(
            tensor=tile_ap.tensor,
            offset=tile_ap.offset + row0 * rs + col0 * cs,
            ap=[[rs * step, n], [cs, ncols]],
        )

    nc = bacc.Bacc(None, target_bir_lowering=False)

    xkT = nc.dram_tensor("xkT", [D, S], bf16, kind="ExternalInput")
    xqT = nc.dram_tensor("xqT", [D, QCHUNK], bf16, kind="ExternalInput")
    # wpack: cols 0:128 wq (pre-scaled), 128:256 wk, 256:384 wv
    wpack = nc.dram_tensor("wpack", [D, 3 * 128], bf16, kind="ExternalInput")
    # woT[p, r, :]: rows 0:32 head 2p, rows 64:96 head 2p+1 (0.5*W_O.T slices)
    woT = nc.dram_tensor("woT", [2, 128, D], bf16, kind="ExternalInput")
    # cpack: cols 0:128 bf16 hi of blend constant c, 128:256 lo residual
    cpack = nc.dram_tensor("cpack", [1, 2 * D], bf16, kind="ExternalInput")
    out = nc.dram_tensor("out", [D, QCHUNK], f32, kind="ExternalOutput")

    with tile.TileContext(nc) as tc, ExitStack() as ctx:
        if loop_n > 1:
            ctx.enter_context(tc.For_i(0, loop_n, 1))
        const = ctx.enter_context(tc.tile_pool(name="const", bufs=1))
        work = ctx.enter_context(tc.tile_pool(name="work", bufs=1))

        # ---- constants / persistent SBUF tensors (batched DMAs) ----
        w_sb = const.tile([128, 3 * 128], bf16, tag="w_sb")
        nc.sync.dma_start(out=w_sb, in_=wpack[:, :])
        wq, wk, wv = (w_sb[:, 128 * i : 128 * (i + 1)] for i in range(3))
        xq_sb = const.tile([128, QCHUNK], bf16, tag="xq_sb")
        for half in range(2):
            sl = slice(half * 512, (half + 1) * 512)
            nc.sync.dma_start(out=xq_sb[:, sl], in_=xqT[:, sl])
        wo_sb = const.tile([128, 2, 128], bf16, tag="wo_sb")
        src = bass.AP(tensor=woT, offset=0, ap=[[128, 128], [128 * 128, 2], [1, 128]])
        nc.sync.dma_start(out=wo_sb, in_=src)
        c_sb = const.tile([1, 2 * D], bf16, tag="c_sb")
        nc.sync.dma_start(out=c_sb, in_=cpack[:, :])
        xk_sb = const.tile([128, S], bf16, tag="xk_sb")
        for half in range(2):
            sl = slice(half * 2048, (half + 1) * 2048)
            nc.gpsimd.dma_start(out=xk_sb[:, sl], in_=xkT[:, sl])

        ones1 = const.tile([1, 512], bf16, tag="ones1")
        nc.vector.memset(ones1, 1.0)
        ones_bc = const.tile([128, 32], bf16, tag="ones_bc")
        nc.vector.memset(ones_bc, 1.0)
        zrow = const.tile([1, 512], bf16, tag="zrow")
        nc.vector.memset(zrow, 0.0)

        kT_sb = const.tile([128, S], bf16, tag="kT_sb")
        qT_sb = const.tile([128, QCHUNK], bf16, tag="qT_sb")
        # v_sb[key, kb, head, 0:32]=v, [...,32]=1.0, [...,33:64]=junk
        # (PSUM rows 33:64/97:128 that the junk feeds are never read)
        v_sb = const.tile([128, NKB, H, 64], bf16, tag="v_sb")
        nc.vector.memset(v_sb[:, :, :, 32], 1.0)

        # ---- prologue: q projections + key chunk 0 (chunks 1-7 are
        # interleaved into the qh0 rounds via st-ring PSUM tiles) ----
        def chunk_mms(pk_ap, pv_ap, c8):
            sl = slice(c8 * 512, (c8 + 1) * 512)
            nc.tensor.matmul(pk_ap, wk, xk_sb[:, sl], start=True, stop=True)
            for j in range(4):
                kb = c8 * 4 + j
                ksl = slice(kb * 128, (kb + 1) * 128)
                nc.tensor.matmul(pv_ap[:, j * 128 : (j + 1) * 128],
                                 xk_sb[:, ksl], wv, start=True, stop=True)

        def chunk_copies(pk_ap, pv_ap, c8):
            sl = slice(c8 * 512, (c8 + 1) * 512)
            kslb = slice(c8 * 4, (c8 + 1) * 4)
            pv4 = pv_ap.rearrange("p (k i h) -> p k i h", k=4, i=H)
            if c8 % 2 == 0:
                nc.scalar.copy(out=kT_sb[:, sl], in_=pk_ap)
                nc.vector.tensor_copy(out=v_sb[:, kslb, :, 0:32], in_=pv4)
            else:
                nc.vector.tensor_copy(out=kT_sb[:, sl], in_=pk_ap)
                nc.scalar.copy(out=v_sb[:, kslb, :, 0:32], in_=pv4)

        with tc.tile_pool(name="proj_ps", bufs=1, space="PSUM") as proj_ps:
            for qh in range(2):
                sl = slice(qh * 512, (qh + 1) * 512)
                pq = proj_ps.tile([128, 512], f32, tag="pk", bufs=2, name="pq")
                nc.tensor.matmul(pq, wq, xq_sb[:, sl], start=True, stop=True)
                nc.vector.tensor_copy(out=qT_sb[:, sl], in_=pq)
            p0 = proj_ps.tile([128, 1024], f32, tag="p0", name="p0")
            chunk_mms(p0[:, 0:512], p0[:, 512:1024], 0)
            chunk_copies(p0[:, 0:512], p0[:, 512:1024], 0)

        # ---- main rounds: scores^T -> exp -> z^T accumulation ----
        zden_ps = ctx.enter_context(tc.tile_pool(name="zden_ps", bufs=1, space="PSUM"))
        round_ctx = ExitStack()
        st_ps = round_ctx.enter_context(
            tc.tile_pool(name="st_ps", bufs=3, space="PSUM"))
        exp_pool = round_ctx.enter_context(tc.tile_pool(name="exp_pool", bufs=2))

        dram_pool = ctx.enter_context(
            tc.tile_pool(name="dram_pool", bufs=1, space="DRAM")
        )
        den_dram = [
            dram_pool.tile([4, 512], f32, tag=f"dd_{qh}", name=f"dd_{qh}")
            for qh in range(2)
        ]
        rec_dram = [
            dram_pool.tile([128, 16], f32, tag=f"rd_{qh}", name=f"rd_{qh}")
            for qh in range(2)
        ]
        den_sb = work.tile([128, 1024], f32, tag="den_sb")
        zc_sb = [work.tile([128, 512], f32, tag=f"zc_{p}", name=f"zc_{p}")
                 for p in range(2)]
        tln_sb = work.tile([128, 1024], f32, tag="tln_sb")
        recrow_sb = work.tile([128, 1024], bf16, tag="recrow_sb")
        rec16 = [work.tile([128, 16], f32, tag=f"rec16_{qh}", name=f"rec16_{qh}")
                 for qh in range(2)]
        rep = [work.tile([128, 512], f32, tag=f"rep_{p}", name=f"rep_{p}")
               for p in range(2)]
        zT_sb = [work.tile([128, QCHUNK], bf16, tag=f"zT_{p}", name=f"zT_{p}")
                 for p in range(2)]

        # z/denominator accumulators: [pair] -> [128, 512] for the current
        # q-half; rows 0:32 z of head 2p, row 32 its denom, rows 64:96 z of
        # head 2p+1, row 96 its denom. qh1 reuses qh0's banks (bufs=1 tags)
        # once qh0's normalization has read them.
        z_cur = [None, None]

        def start_qh():
            for p in range(2):
                z_cur[p] = zden_ps.tile(
                    [128, 512], f32, tag=f"z_{p}", name=f"z_{p}"
                )
                nc.tensor.matmul(
                    z_cur[p], zrow[:, 0:128], zrow, start=True, stop=False,
                    skip_group_check=True,
                )

        def emit_scores(qh, kb):
            qsl = slice(qh * 512, (qh + 1) * 512)
            ksl = slice(kb * 128, (kb + 1) * 128)
            sts = []
            for p in range(2):
                st = st_ps.tile([128, 1024], f32, tag="st", name=f"st_{p}")
                for j in range(2):
                    i = 2 * p + j
                    nc.tensor.matmul(
                        st[:, j * 512 : (j + 1) * 512],
                        kT_sb[32 * i : 32 * (i + 1), ksl],
                        qT_sb[32 * i : 32 * (i + 1), qsl],
                        start=True,
                        stop=True,
                        tile_position=(32 * i, 0),
                    )
                sts.append(st)
            return sts

        def emit_exp(sts, engines):
            ex = [None, None]
            for p in range(2):
                st = sts[p]
                e = exp_pool.tile([128, 1024], bf16, tag=f"ex_{p}", name=f"ex_{p}")
                if engines[p] == "S":
                    nc.scalar.activation(
                        out=e, in_=bf16_hi_alias(st[:, :]), func=AF.Exp,
                        scale=ACT_SCALE,
                    )
                else:
                    src = bf16_hi_alias(st[:, :]) if ve_hi else st[:, :]
                    nc.vector._custom_dve(
                        exp2_op, out=i16_alias(e[:, :]), in0=src,
                        s0=1.0 / 256.0, s1=(127.0 - EXP2_SIGMA) * 128.0,
                    )
                ex[p] = e
            return ex

        def emit_z(kb, ex):
            for p in range(2):
                for j in range(2):
                    nc.tensor.matmul(
                        z_cur[p][64 * j : 64 * j + 64, :],
                        v_sb[:, kb, 2 * p + j, :],
                        ex[p][:, j * 512 : (j + 1) * 512],
                        start=False,
                        stop=(kb == NKB - 1),
                        tile_position=(0, 64 * j),
                        skip_group_check=True,
                    )

        def emit_epilogue_tail(u_ps):
            # qh1 (tail) epilogue: latency-optimized, no DRAM bounces.
            # 1/d = exp(-ln d) per denom row on ScalarE (idle at the tail),
            # broadcast over 32 z rows via a K=1 TensorE matmul, normalize.
            qsl = slice(512, 1024)
            for p in range(2):
                nc.vector.tensor_copy(out=zc_sb[p], in_=z_cur[p])
            rec_rep = [u_ps.tile([128, 512], f32, tag=f"rr_{p}", bufs=1,
                               name=f"rr_{p}")
                       for p in range(2)]
            for h in (0, 2, 1, 3):
                p, j = h // 2, h % 2
                r = 64 * j + 32
                csl = slice(p * 512, (p + 1) * 512)
                nc.scalar.activation(out=tln_sb[r : r + 1, csl],
                                     in_=z_cur[p][r : r + 1, :], func=AF.Ln)
                nc.scalar.activation(out=recrow_sb[r : r + 1, csl],
                                     in_=tln_sb[r : r + 1, csl], func=AF.Exp,
                                     scale=-1.0)
                nc.tensor.matmul(
                    rec_rep[p][64 * j : 64 * j + 32, :],
                    ones_bc[r : r + 1, :],
                    recrow_sb[r : r + 1, csl],
                    start=True, stop=True, skip_group_check=True,
                    tile_position=(r, 64 * j),
                )
                rsl = slice(64 * j, 64 * j + 32)
                nc.vector.tensor_mul(
                    zT_sb[p][rsl, qsl], zc_sb[p][rsl, :], rec_rep[p][rsl, :]
                )

        def emit_epilogue(qh):
            # per-qh normalization, overlapped with the next qh's rounds:
            # denom rows (PSUM partitions 32/96) -> SBUF -> DRAM -> reshaped
            # [128,16] for a cheap wide reciprocal -> DRAM -> partition-
            # broadcast back over the z rows.
            qsl = slice(qh * 512, (qh + 1) * 512)
            # free the z PSUM banks ASAP (the next q-half's zeroing matmul
            # WAR-waits on all reads): pull z rows + denom rows into SBUF
            # with one [128,512] copy per pair, then normalize from SBUF.
            for p in range(2):
                if p == 0:
                    nc.scalar.copy(out=zc_sb[p], in_=z_cur[p])
                else:
                    nc.vector.tensor_copy(out=zc_sb[p], in_=z_cur[p])
            for p in range(2):
                for j in range(2):
                    r = 64 * j + 32
                    csl = slice(p * 512, (p + 1) * 512)
                    if p == 0:
                        nc.scalar.copy(out=den_sb[r : r + 1, csl],
                                       in_=zc_sb[p][r : r + 1, :])
                    else:
                        nc.vector.tensor_copy(out=den_sb[r : r + 1, csl],
                                              in_=zc_sb[p][r : r + 1, :])
            # den_dram rows: h = 2*p + j  <- den_sb row 32+64j, cols p*512
            dd = den_dram[qh]
            for j in range(2):
                r = 64 * j + 32
                dst = bass.AP(tensor=dd.tensor, offset=dd.offset + j * 512,
                              ap=[[1024, 2], [1, 512]])  # (p, q)
                nc.sync.dma_start(out=dst, in_=den_sb[r : r + 1, 0:1024])
            # gather all 4 heads' denoms as [128, 16]
            gsrc = bass.AP(tensor=dd.tensor, offset=dd.offset,
                           ap=[[16, 128], [1, 16]])
            nc.sync.dma_start(out=rec16[qh], in_=gsrc)
            nc.vector.reciprocal(out=rec16[qh], in_=rec16[qh])
            nc.sync.dma_start(out=rec_dram[qh], in_=rec16[qh])
            for p in range(2):
                # rep[p] rows 0:32 <- head 2p, rows 64:96 <- head 2p+1
                for j in range(2):
                    h = 2 * p + j
                    srcap = bass.AP(tensor=rec_dram[qh].tensor,
                                    offset=rec_dram[qh].offset + h * 512,
                                    ap=[[0, 32], [1, 512]])
                    nc.sync.dma_start(out=rep[p][64 * j : 64 * j + 32, :],
                                      in_=srcap)
                    rsl = slice(64 * j, 64 * j + 32)
                    nc.vector.tensor_mul(
                        zT_sb[p][rsl, qsl], zc_sb[p][rsl, :], rep[p][rsl, :]
                    )

        # engine assignment: p0 -> ScalarE, p1 -> VectorE; optionally every
        # flip_every-th round sends p1 to ScalarE too (rebalance knob).
        def engines_for(k):
            if flip_every and k % flip_every == flip_every - 1:
                return ("S", "S")
            return ("S", "V")

        for qh in range(2):
            start_qh()
            sts = emit_scores(qh, 0)
            for kb in range(NKB):
                ex = emit_exp(sts, engines_for(qh * NKB + kb))
                if qh == 0 and kb % 4 == 1 and kb // 4 < 7:
                    c8 = kb // 4 + 1
                    ct = st_ps.tile([128, 1024], f32, tag="st", name=f"ck_{c8}")
                    chunk_mms(ct[:, 0:512], ct[:, 512:1024], c8)
                    cur_chunk = ct
                elif qh == 0 and kb % 4 == 2 and kb // 4 < 7:
                    c8 = kb // 4 + 1
                    sl = slice(c8 * 512, (c8 + 1) * 512)
                    if c8 % 2 == 0:
                        nc.scalar.copy(out=kT_sb[:, sl], in_=cur_chunk[:, 0:512])
                    else:
                        nc.vector.tensor_copy(out=kT_sb[:, sl],
                                              in_=cur_chunk[:, 0:512])
                elif qh == 0 and kb % 4 == 3 and kb // 4 < 7:
                    c8 = kb // 4 + 1
                    kslb = slice(c8 * 4, (c8 + 1) * 4)
                    pv4 = cur_chunk[:, 512:1024].rearrange(
                        "p (k i h) -> p k i h", k=4, i=H)
                    if c8 % 2 == 0:
                        nc.vector.tensor_copy(out=v_sb[:, kslb, :, 0:32], in_=pv4)
                    else:
                        nc.scalar.copy(out=v_sb[:, kslb, :, 0:32], in_=pv4)
                if kb + 1 < NKB:
                    sts = emit_scores(qh, kb + 1)
                emit_z(kb, ex)
            if qh == 0:
                emit_epilogue(qh)

        round_ctx.close()

        # ---- final projection: out^T[d, q] per q-half (W_O stationary,
        # z^T moving at N=512); blend constant via two K=1 bf16 matmuls.
        # qh0's combine is VectorE-only so it overlaps the qh1 tail
        # epilogue's ScalarE ln/exp chain. ----
        with tc.tile_pool(name="u_ps", bufs=2, space="PSUM") as u_ps, tc.tile_pool(
            name="out_pool", bufs=2
        ) as out_pool:
            emit_epilogue_tail(u_ps)
            for qh in range(2):
                qsl = slice(qh * 512, (qh + 1) * 512)
                ue = u_ps.tile([128, 512], f32, tag="ue")  # heads at rows 0:32
                uo = u_ps.tile([128, 512], f32, tag="uo")  # heads at rows 64:96
                nc.tensor.matmul(
                    ue, wo_sb[0:32, 0, :], zT_sb[0][0:32, qsl], start=True,
                    stop=False, skip_group_check=True, tile_position=(0, 0),
                )
                nc.tensor.matmul(
                    ue, wo_sb[0:32, 1, :], zT_sb[1][0:32, qsl], start=False,
                    stop=False, skip_group_check=True, tile_position=(0, 0),
                )
                nc.tensor.matmul(
                    ue, c_sb[0:1, 0:128], ones1, start=False, stop=False,
                    skip_group_check=True, tile_position=(0, 0),
                )
                nc.tensor.matmul(
                    ue, c_sb[0:1, 128:256], ones1, start=False, stop=True,
                    skip_group_check=True, tile_position=(0, 0),
                )
                nc.tensor.matmul(
                    uo, wo_sb[64:96, 0, :], zT_sb[0][64:96, qsl], start=True,
                    stop=False, skip_group_check=True, tile_position=(64, 0),
                )
                nc.tensor.matmul(
                    uo, wo_sb[64:96, 1, :], zT_sb[1][64:96, qsl], start=False,
                    stop=True, skip_group_check=True, tile_position=(64, 0),
                )
                ob = out_pool.tile([128, 512], f32, tag="ob")
                nc.vector.tensor_copy(out=ob, in_=ue)
                nc.vector.tensor_add(ob, ob, uo)
                q = nc.sync if qh == 0 else nc.gpsimd
                q.dma_start(out=out[:, qsl], in_=ob)

    nc.compile()
    return nc


def _get_program(loop_n: int = 1):
    import os

    global _PROGRAM
    ve_hi = os.environ.get("BASS_VE_HI", "1") == "1"
    flip = int(os.environ.get("BASS_FLIP", "10"))
    if loop_n != 1:
        return _build_program(loop_n, ve_hi, flip)
    if _PROGRAM is None:
        _PROGRAM = _build_program(1, ve_hi, flip)
    return _PROGRAM


def make_in_maps(x, W_K, W_Q, W_V, W_O):
    x = np.asarray(x, np.float32)
    W_K = np.asarray(W_K, np.float32)
    W_Q = np.asarray(W_Q, np.float32)
    W_V = np.asarray(W_V, np.float32)
    W_O = np.asarray(W_O, np.float32)

    wqT = np.ascontiguousarray((W_Q.transpose(2, 0, 1).reshape(D, H * DH)) * PRESCALE)
    wkT = np.ascontiguousarray(W_K.transpose(2, 0, 1).reshape(D, H * DH))
    wvT = np.ascontiguousarray(W_V.transpose(2, 0, 1).reshape(D, H * DH))
    wpack = np.concatenate([wqT, wkT, wvT], axis=1).astype(BF16)
    woT_flat = 0.5 * W_O.T  # [f, d']
    woT = np.zeros((2, 128, D), np.float32)
    for p in range(2):
        woT[p, 0:32] = woT_flat[(2 * p) * 32 : (2 * p) * 32 + 32]
        woT[p, 64:96] = woT_flat[(2 * p + 1) * 32 : (2 * p + 1) * 32 + 32]

    in_maps = []
    for core in range(NCORES):
        b, qc = divmod(core, 4)
        xb = x[b]
        xkT_b = np.ascontiguousarray(xb.T).astype(BF16)
        xqT_c = np.ascontiguousarray(xb[qc * QCHUNK : (qc + 1) * QCHUNK].T).astype(BF16)
        # exact blend constant: c = 0.5 * (sum_k v[k]) @ W_O^T, split into
        # bf16 hi + lo for two exact-ish K=1 matmuls
        sv = (xb.sum(0, dtype=np.float64) @ wvT.astype(np.float64))
        c = (0.5 * (sv @ W_O.T.astype(np.float64))).astype(np.float32)
        c_hi = c.astype(BF16)
        c_lo = (c - c_hi.astype(np.float32)).astype(BF16)
        cpack = np.concatenate([c_hi, c_lo])[None, :].astype(BF16)
        in_maps.append(
            {
                "xkT": xkT_b,
                "xqT": xqT_c,
                "wpack": wpack,
                "woT": woT.astype(BF16),
                "cpack": cpack,
            }
        )
    return in_maps


def kernel(x, W_K, W_Q, W_V, W_O):
    from concourse.bass_utils import run_bass_kernel_spmd

    nc = _get_program()
    in_maps = make_in_maps(x, W_K, W_Q, W_V, W_O)
    res = run_bass_kernel_spmd(nc, in_maps, core_ids=list(range(NCORES)))
    full = np.empty((B, S, D), np.float32)
    for core in range(NCORES):
        b, qc = divmod(core, 4)
        full[b, qc * QCHUNK : (qc + 1) * QCHUNK, :] = res.results[core]["out"].T
    return full


# revision 19
# speedup vs baseline: 1.0590x; 1.0079x over previous
"""Trainium2 Bass kernel for a 4-head attention layer with post-softmax
affine blend (attn = 0.5*softmax(qk/sqrt(dh)) + 0.5), distributed over 8
NeuronCores.

Reference computation (B=2, S=4096, D=128, H=4, Dh=32):
    k = einsum('ihd,bpd->biph', W_K, x)
    q = einsum('ihd,bpd->biph', W_Q, x)
    v = einsum('ihd,bpd->biph', W_V, x)
    scores = einsum('biph,biqh->biqp', k, q) / sqrt(32)
    attn   = softmax(scores, -1) * 0.5 + 0.5
    z      = einsum('biph,biqp->biqh', v, attn)
    out    = einsum('df,bpf->bpd', W_O, z_flat)

Sharding: 8 cores = (batch b in {0,1}) x (query chunk qc in 4 x 1024).
Each core computes all 4 heads for its 1024 queries against all 4096
keys and emits the disjoint output slice out[b, qc*1024:(qc+1)*1024, :]
(transposed on-chip as [D, 1024]; the host unshard transposes back).

Per-core algorithm (everything stays on-chip):
  - Prologue: batched input DMAs split over two queues, then a dense
    projection burst (k^T/q^T/v on TensorE) that also un-throttles the
    PE HAM clock gate.
  - Main rounds (2 q-halves x 32 key-blocks): scores^T tiles
    [128 keys x 2x512 q] via 4 row-packed K=32 matmuls into a 3-deep
    PSUM ring; exp is computed from PSUM split across ScalarE
    (activation, scale folded into W_Q) and VectorE (Schraudolph exp2
    custom DVE op writing bf16 bits via int16); attn@v accumulates
    z^T in PSUM with col-packed M=64 matmuls whose stationary operand
    [v_i | ones | 0] also accumulates the softmax denominator.
  - Epilogue per q-half: denominators are reshaped through DRAM onto
    128 partitions for a cheap VectorE reciprocal, broadcast back, and
    applied to z^T.
  - Final projection is transposed (W_O slices stationary, z^T moving,
    N=512): out^T accumulates in PSUM; the uniform 0.5*sum_k(v) blend
    constant is added via two K=1 bf16 hi/lo matmuls.
"""

import math

import numpy as np
import ml_dtypes

BF16 = ml_dtypes.bfloat16

B, S, D, H, DH = 2, 4096, 128, 4, 32
QCHUNK = 1024  # queries per core
NCORES = 8
NKB = S // 128  # 32 key blocks
# exp(s) is computed as exp((s * 2^15 * log2(e)) * ln(2) / 2^15); the big
# pre-scale is folded into W_Q so a bit-trick exp2 on VectorE can share the
# same score tensor.
PRESCALE = (2.0**15) * math.log2(math.e) / math.sqrt(DH)
ACT_SCALE = math.log(2.0) / (2.0**15)

# Schraudolph exp2 constant: sigma balances the multiplicative error of the
# linear-mantissa approximation; folded into the int16 bf16-bit construction.
EXP2_SIGMA = 0.02979

_PROGRAM = None


def _register_exp2():
    """Register (once) a fused y = x*C0 + C1 custom DVE op whose int16
    output, reinterpreted as bf16, is 2^(x/2^15) a la Schraudolph."""
    from concourse import dve_ops
    from concourse.dve_spec import Spec, Src0, C0, C1, lower, _has_src1
    from concourse.dve_uop import DveOpSpec

    name = "EXP2_SCHRAU_ANT"
    for o in dve_ops.OPS:
        if o.name == name:
            return o
    spec = Spec(body=Src0 * C0 + C1,
                reference=lambda in0, in1, c0, c1, c2: in0 * c0 + c1)
    opcode = dve_ops._CUSTOM_DVE_ROW_BASE + len(dve_ops.OPS)
    shas = {}
    for ver in ("v3", "v4"):
        s = DveOpSpec(name=name, opcode=opcode, uops=lower(spec, ver=ver),
                      rd1_en=_has_src1(spec))
        shas[ver] = s.sha(ver)
    op = dve_ops.DveOp(name, spec, subdim=False, uops_sha=shas)
    dve_ops.OPS.append(op)
    dve_ops.CUSTOM_DVE_SPECS[name] = spec
    dve_ops._SUB_OPCODE_FOR_NAME[name] = opcode
    return op


def _build_program(loop_n: int = 1, ve_hi: bool = True, flip_every: int = 10):
    import concourse.bass as bass
    import concourse.mybir as mybir
    import concourse.tile as tile
    from concourse import bacc
    from contextlib import ExitStack

    import dataclasses

    f32 = mybir.dt.float32
    bf16 = mybir.dt.bfloat16
    AF = mybir.ActivationFunctionType
    exp2_op = _register_exp2()

    def i16_alias(ap):
        h = dataclasses.replace(ap.tensor, dtype=mybir.dt.int16)
        return bass.AP(tensor=h, offset=ap.offset, ap=[list(d) for d in ap.ap])

    def bf16_hi_alias(ap):
        """View an f32 [P, N] AP as the bf16 high halves: [P, N] bf16,
        element stride 2, offset +1 (little-endian high 2 bytes)."""
        h = dataclasses.replace(
            ap.tensor, dtype=mybir.dt.bfloat16,
            shape=[ap.tensor.shape[0], ap.tensor.shape[1] * 2],
        )
        newap = [[ap.ap[0][0] * 2, ap.ap[0][1]]] + [
            [d[0] * 2, d[1]] for d in ap.ap[1:]
        ]
        return bass.AP(tensor=h, offset=ap.offset * 2 + 1, ap=newap)

    def pstride(tile_ap, row0, step, n, col0, ncols):
        """Partition-strided view of a [128, C] tile: rows row0, row0+step,
        ... (n of them), cols col0:col0+ncols."""
        rs = tile_ap.ap[0][0]
        cs = tile_ap.ap[1][0]
        return bass.AP(
            tensor=tile_ap.tensor,
            offset=tile_ap.offset + row0 * rs + col0 * cs,
            ap=[[rs * step, n], [cs, ncols]],
        )

    nc = bacc.Bacc(None, target_bir_lowering=False)

    xkT = nc.dram_tensor("xkT", [D, S], bf16, kind="ExternalInput")
    xqT = nc.dram_tensor("xqT", [D, QCHUNK], bf16, kind="ExternalInput")
    # wpack: cols 0:128 wq (pre-scaled), 128:256 wk, 256:384 wv
    wpack = nc.dram_tensor("wpack", [D, 3 * 128], bf16, kind="ExternalInput")
    # woT[p, r, :]: rows 0:32 head 2p, rows 64:96 head 2p+1 (0.5*W_O.T slices)
    woT = nc.dram_tensor("woT", [2, 128, D], bf16, kind="ExternalInput")
    # cpack: cols 0:128 bf16 hi of blend constant c, 128:256 lo residual
    cpack = nc.dram_tensor("cpack", [1, 2 * D], bf16, kind="ExternalInput")
    out = nc.dram_tensor("out", [D, QCHUNK], f32, kind="ExternalOutput")

    with tile.TileContext(nc) as tc, ExitStack() as ctx:
        if loop_n > 1:
            ctx.enter_context(tc.For_i(0, loop_n, 1))
        const = ctx.enter_context(tc.tile_pool(name="const", bufs=1))
        work = ctx.enter_context(tc.tile_pool(name="work", bufs=1))

        # ---- constants / persistent SBUF tensors (batched DMAs) ----
        w_sb = const.tile([128, 3 * 128], bf16, tag="w_sb")
        nc.sync.dma_start(out=w_sb, in_=wpack[:, :])
        wq, wk, wv = (w_sb[:, 128 * i : 128 * (i + 1)] for i in range(3))
        xq_sb = const.tile([128, QCHUNK], bf16, tag="xq_sb")
        for half in range(2):
            sl = slice(half * 512, (half + 1) * 512)
            nc.sync.dma_start(out=xq_sb[:, sl], in_=xqT[:, sl])
        xk_sb = const.tile([128, S], bf16, tag="xk_sb")
        for half in range(2):
            sl = slice(half * 2048, (half + 1) * 2048)
            nc.sync.dma_start(out=xk_sb[:, sl], in_=xkT[:, sl])
        wo_sb = const.tile([128, 2, 128], bf16, tag="wo_sb")
        src = bass.AP(tensor=woT, offset=0, ap=[[128, 128], [128 * 128, 2], [1, 128]])
        nc.gpsimd.dma_start(out=wo_sb, in_=src)
        c_sb = const.tile([1, 2 * D], bf16, tag="c_sb")
        nc.gpsimd.dma_start(out=c_sb, in_=cpack[:, :])

        ones1 = const.tile([1, 512], bf16, tag="ones1")
        nc.vector.memset(ones1, 1.0)
        ones_bc = const.tile([128, 32], bf16, tag="ones_bc")
        nc.vector.memset(ones_bc, 1.0)
        zrow = const.tile([1, 512], bf16, tag="zrow")
        nc.vector.memset(zrow, 0.0)

        kT_sb = const.tile([128, S], bf16, tag="kT_sb")
        qT_sb = const.tile([128, QCHUNK], bf16, tag="qT_sb")
        # v_sb[key, kb, head, 0:32]=v, [...,32]=1.0, [...,33:64]=junk
        # (PSUM rows 33:64/97:128 that the junk feeds are never read)
        v_sb = const.tile([128, NKB, H, 64], bf16, tag="v_sb")
        nc.vector.memset(v_sb[:, :, :, 32], 1.0)

        # ---- prologue: q projections + key chunk 0 (chunks 1-7 are
        # interleaved into the qh0 rounds via st-ring PSUM tiles) ----
        def chunk_mms(pk_ap, pv_ap, c8):
            sl = slice(c8 * 512, (c8 + 1) * 512)
            nc.tensor.matmul(pk_ap, wk, xk_sb[:, sl], start=True, stop=True)
            for j in range(4):
                kb = c8 * 4 + j
                ksl = slice(kb * 128, (kb + 1) * 128)
                nc.tensor.matmul(pv_ap[:, j * 128 : (j + 1) * 128],
                                 xk_sb[:, ksl], wv, start=True, stop=True)

        def chunk_copies(pk_ap, pv_ap, c8):
            sl = slice(c8 * 512, (c8 + 1) * 512)
            kslb = slice(c8 * 4, (c8 + 1) * 4)
            pv4 = pv_ap.rearrange("p (k i h) -> p k i h", k=4, i=H)
            if c8 % 2 == 0:
                nc.scalar.copy(out=kT_sb[:, sl], in_=pk_ap)
                nc.vector.tensor_copy(out=v_sb[:, kslb, :, 0:32], in_=pv4)
            else:
                nc.vector.tensor_copy(out=kT_sb[:, sl], in_=pk_ap)
                nc.scalar.copy(out=v_sb[:, kslb, :, 0:32], in_=pv4)

        with tc.tile_pool(name="proj_ps", bufs=1, space="PSUM") as proj_ps:
            for qh in range(2):
                sl = slice(qh * 512, (qh + 1) * 512)
                pq = proj_ps.tile([128, 512], f32, tag="pk", bufs=2, name="pq")
                nc.tensor.matmul(pq, wq, xq_sb[:, sl], start=True, stop=True)
                nc.vector.tensor_copy(out=qT_sb[:, sl], in_=pq)
            p0 = proj_ps.tile([128, 1024], f32, tag="p0", name="p0")
            chunk_mms(p0[:, 0:512], p0[:, 512:1024], 0)
            chunk_copies(p0[:, 0:512], p0[:, 512:1024], 0)

        # ---- main rounds: scores^T -> exp -> z^T accumulation ----
        zden_ps = ctx.enter_context(tc.tile_pool(name="zden_ps", bufs=1, space="PSUM"))
        round_ctx = ExitStack()
        st_ps = round_ctx.enter_context(
            tc.tile_pool(name="st_ps", bufs=3, space="PSUM"))
        exp_pool = round_ctx.enter_context(tc.tile_pool(name="exp_pool", bufs=2))

        dram_pool = ctx.enter_context(
            tc.tile_pool(name="dram_pool", bufs=1, space="DRAM")
        )
        den_dram = [
            dram_pool.tile([4, 512], f32, tag=f"dd_{qh}", name=f"dd_{qh}")
            for qh in range(2)
        ]
        rec_dram = [
            dram_pool.tile([128, 16], f32, tag=f"rd_{qh}", name=f"rd_{qh}")
            for qh in range(2)
        ]
        den_sb = work.tile([128, 1024], f32, tag="den_sb")
        zc_sb = [work.tile([128, 512], f32, tag=f"zc_{p}", name=f"zc_{p}")
                 for p in range(2)]
        tln_sb = work.tile([128, 1024], f32, tag="tln_sb")
        recrow_sb = work.tile([128, 1024], bf16, tag="recrow_sb")
        rec16 = [work.tile([128, 16], f32, tag=f"rec16_{qh}", name=f"rec16_{qh}")
                 for qh in range(2)]
        rep = [work.tile([128, 512], f32, tag=f"rep_{p}", name=f"rep_{p}")
               for p in range(2)]
        zT_sb = [work.tile([128, QCHUNK], bf16, tag=f"zT_{p}", name=f"zT_{p}")
                 for p in range(2)]

        # z/denominator accumulators: [pair] -> [128, 512] for the current
        # q-half; rows 0:32 z of head 2p, row 32 its denom, rows 64:96 z of
        # head 2p+1, row 96 its denom. qh1 reuses qh0's banks (bufs=1 tags)
        # once qh0's normalization has read them.
        z_cur = [None, None]

        def start_qh():
            for p in range(2):
                z_cur[p] = zden_ps.tile(
                    [128, 512], f32, tag=f"z_{p}", name=f"z_{p}"
                )
                nc.tensor.matmul(
                    z_cur[p], zrow[:, 0:128], zrow, start=True, stop=False,
                    skip_group_check=True,
                )

        def emit_scores(qh, kb):
            qsl = slice(qh * 512, (qh + 1) * 512)
            ksl = slice(kb * 128, (kb + 1) * 128)
            sts = []
            for p in range(2):
                st = st_ps.tile([128, 1024], f32, tag="st", name=f"st_{p}")
                for j in range(2):
                    i = 2 * p + j
                    nc.tensor.matmul(
                        st[:, j * 512 : (j + 1) * 512],
                        kT_sb[32 * i : 32 * (i + 1), ksl],
                        qT_sb[32 * i : 32 * (i + 1), qsl],
                        start=True,
                        stop=True,
                        tile_position=(32 * i, 0),
                    )
                sts.append(st)
            return sts

        def emit_exp(sts, engines):
            ex = [None, None]
            for p in range(2):
                st = sts[p]
                e = exp_pool.tile([128, 1024], bf16, tag=f"ex_{p}", name=f"ex_{p}")
                if engines[p] == "S":
                    nc.scalar.activation(
                        out=e, in_=bf16_hi_alias(st[:, :]), func=AF.Exp,
                        scale=ACT_SCALE,
                    )
                else:
                    src = bf16_hi_alias(st[:, :]) if ve_hi else st[:, :]
                    nc.vector._custom_dve(
                        exp2_op, out=i16_alias(e[:, :]), in0=src,
                        s0=1.0 / 256.0, s1=(127.0 - EXP2_SIGMA) * 128.0,
                    )
                ex[p] = e
            return ex

        def emit_z(kb, ex):
            for p in range(2):
                for j in range(2):
                    nc.tensor.matmul(
                        z_cur[p][64 * j : 64 * j + 64, :],
                        v_sb[:, kb, 2 * p + j, :],
                        ex[p][:, j * 512 : (j + 1) * 512],
                        start=False,
                        stop=(kb == NKB - 1),
                        tile_position=(0, 64 * j),
                        skip_group_check=True,
                    )

        def emit_epilogue_tail(u_ps):
            # qh1 (tail) epilogue: latency-optimized, no DRAM bounces.
            # 1/d = exp(-ln d) per denom row on ScalarE (idle at the tail),
            # broadcast over 32 z rows via a K=1 TensorE matmul, normalize.
            qsl = slice(512, 1024)
            for p in range(2):
                nc.vector.tensor_copy(out=zc_sb[p], in_=z_cur[p])
            rec_rep = [u_ps.tile([128, 512], f32, tag=f"rr_{p}", bufs=1,
                               name=f"rr_{p}")
                       for p in range(2)]
            order = (0, 2, 1, 3)
            for h in order:  # all Lns first: one act-table set switch total
                p, j = h // 2, h % 2
                r = 64 * j + 32
                csl = slice(p * 512, (p + 1) * 512)
                nc.scalar.activation(out=tln_sb[r : r + 1, csl],
                                     in_=z_cur[p][r : r + 1, :], func=AF.Ln)
            for h in order:
                p, j = h // 2, h % 2
                r = 64 * j + 32
                csl = slice(p * 512, (p + 1) * 512)
                nc.scalar.activation(out=recrow_sb[r : r + 1, csl],
                                     in_=tln_sb[r : r + 1, csl], func=AF.Exp,
                                     scale=-1.0)
                nc.tensor.matmul(
                    rec_rep[p][64 * j : 64 * j + 32, :],
                    ones_bc[r : r + 1, :],
                    recrow_sb[r : r + 1, csl],
                    start=True, stop=True, skip_group_check=True,
                    tile_position=(r, 64 * j),
                )
                rsl = slice(64 * j, 64 * j + 32)
                nc.vector.tensor_mul(
                    zT_sb[p][rsl, qsl], zc_sb[p][rsl, :], rec_rep[p][rsl, :]
                )

        def emit_epilogue(qh):
            # per-qh normalization, overlapped with the next qh's rounds:
            # denom rows (PSUM partitions 32/96) -> SBUF -> DRAM -> reshaped
            # [128,16] for a cheap wide reciprocal -> DRAM -> partition-
            # broadcast back over the z rows.
            qsl = slice(qh * 512, (qh + 1) * 512)
            # free the z PSUM banks ASAP (the next q-half's zeroing matmul
            # WAR-waits on all reads): pull z rows + denom rows into SBUF
            # with one [128,512] copy per pair, then normalize from SBUF.
            for p in range(2):
                if p == 0:
                    nc.scalar.copy(out=zc_sb[p], in_=z_cur[p])
                else:
                    nc.vector.tensor_copy(out=zc_sb[p], in_=z_cur[p])
            for p in range(2):
                for j in range(2):
                    r = 64 * j + 32
                    csl = slice(p * 512, (p + 1) * 512)
                    if p == 0:
                        nc.scalar.copy(out=den_sb[r : r + 1, csl],
                                       in_=zc_sb[p][r : r + 1, :])
                    else:
                        nc.vector.tensor_copy(out=den_sb[r : r + 1, csl],
                                              in_=zc_sb[p][r : r + 1, :])
            # den_dram rows: h = 2*p + j  <- den_sb row 32+64j, cols p*512
            dd = den_dram[qh]
            for j in range(2):
                r = 64 * j + 32
                dst = bass.AP(tensor=dd.tensor, offset=dd.offset + j * 512,
                              ap=[[1024, 2], [1, 512]])  # (p, q)
                nc.sync.dma_start(out=dst, in_=den_sb[r : r + 1, 0:1024])
            # gather all 4 heads' denoms as [128, 16]
            gsrc = bass.AP(tensor=dd.tensor, offset=dd.offset,
                           ap=[[16, 128], [1, 16]])
            nc.sync.dma_start(out=rec16[qh], in_=gsrc)
            nc.vector.reciprocal(out=rec16[qh], in_=rec16[qh])
            nc.sync.dma_start(out=rec_dram[qh], in_=rec16[qh])
            for p in range(2):
                # rep[p] rows 0:32 <- head 2p, rows 64:96 <- head 2p+1
                for j in range(2):
                    h = 2 * p + j
                    srcap = bass.AP(tensor=rec_dram[qh].tensor,
                                    offset=rec_dram[qh].offset + h * 512,
                                    ap=[[0, 32], [1, 512]])
                    nc.sync.dma_start(out=rep[p][64 * j : 64 * j + 32, :],
                                      in_=srcap)
                    rsl = slice(64 * j, 64 * j + 32)
                    nc.vector.tensor_mul(
                        zT_sb[p][rsl, qsl], zc_sb[p][rsl, :], rep[p][rsl, :]
                    )

        # engine assignment: p0 -> ScalarE, p1 -> VectorE, except:
        # - chunk-projection rounds (qh0, kb%4==1): both exps on VectorE
        #   while ScalarE does the chunk copies
        # - early qh1 rounds: both on ScalarE while VectorE runs the qh0
        #   normalization epilogue
        # - every flip_every-th remaining round: both on ScalarE (balance)
        def engines_for(k):
            if k < NKB and k % 4 == 1 and k // 4 < 7:
                return ("V", "V")
            if NKB <= k < NKB + 4:
                return ("S", "S")
            if flip_every and k % flip_every == flip_every - 1:
                return ("S", "S")
            return ("S", "V")

        for qh in range(2):
            start_qh()
            sts = emit_scores(qh, 0)
            for kb in range(NKB):
                ex = emit_exp(sts, engines_for(qh * NKB + kb))
                if qh == 0 and kb % 4 == 1 and kb // 4 < 7:
                    c8 = kb // 4 + 1
                    ct = st_ps.tile([128, 1024], f32, tag="st", name=f"ck_{c8}")
                    chunk_mms(ct[:, 0:512], ct[:, 512:1024], c8)
                    sl = slice(c8 * 512, (c8 + 1) * 512)
                    kslb = slice(c8 * 4, (c8 + 1) * 4)
                    pv4 = ct[:, 512:1024].rearrange(
                        "p (k i h) -> p k i h", k=4, i=H)
                    nc.scalar.copy(out=kT_sb[:, sl], in_=ct[:, 0:512])
                    nc.scalar.copy(out=v_sb[:, kslb, :, 0:32], in_=pv4)
                if kb + 1 < NKB:
                    sts = emit_scores(qh, kb + 1)
                emit_z(kb, ex)
            if qh == 0:
                emit_epilogue(qh)

        round_ctx.close()

        # ---- final projection: out^T[d, q] per q-half (W_O stationary,
        # z^T moving at N=512); blend constant via two K=1 bf16 matmuls.
        # qh0's combine is VectorE-only so it overlaps the qh1 tail
        # epilogue's ScalarE ln/exp chain. ----
        with tc.tile_pool(name="u_ps", bufs=2, space="PSUM") as u_ps, tc.tile_pool(
            name="out_pool", bufs=2
        ) as out_pool:
            emit_epilogue_tail(u_ps)
            for qh in range(2):
                qsl = slice(qh * 512, (qh + 1) * 512)
                ue = u_ps.tile([128, 512], f32, tag="ue")  # heads at rows 0:32
                uo = u_ps.tile([128, 512], f32, tag="uo")  # heads at rows 64:96
                nc.tensor.matmul(
                    ue, wo_sb[0:32, 0, :], zT_sb[0][0:32, qsl], start=True,
                    stop=False, skip_group_check=True, tile_position=(0, 0),
                )
                nc.tensor.matmul(
                    ue, wo_sb[0:32, 1, :], zT_sb[1][0:32, qsl], start=False,
                    stop=False, skip_group_check=True, tile_position=(0, 0),
                )
                nc.tensor.matmul(
                    ue, c_sb[0:1, 0:128], ones1, start=False, stop=False,
                    skip_group_check=True, tile_position=(0, 0),
                )
                nc.tensor.matmul(
                    ue, c_sb[0:1, 128:256], ones1, start=False, stop=True,
                    skip_group_check=True, tile_position=(0, 0),
                )
                nc.tensor.matmul(
                    uo, wo_sb[64:96, 0, :], zT_sb[0][64:96, qsl], start=True,
                    stop=False, skip_group_check=True, tile_position=(64, 0),
                )
                nc.tensor.matmul(
                    uo, wo_sb[64:96, 1, :], zT_sb[1][64:96, qsl], start=False,
                    stop=True, skip_group_check=True, tile_position=(64, 0),
                )
                ob = out_pool.tile([128, 512], f32, tag="ob")
                nc.vector.tensor_copy(out=ob, in_=ue)
                nc.vector.tensor_add(ob, ob, uo)
                q = nc.sync if qh == 0 else nc.gpsimd
                q.dma_start(out=out[:, qsl], in_=ob)

    nc.compile()
    return nc


def _get_program(loop_n: int = 1):
    import os

    global _PROGRAM
    ve_hi = os.environ.get("BASS_VE_HI", "1") == "1"
    flip = int(os.environ.get("BASS_FLIP", "10"))
    if loop_n != 1:
        return _build_program(loop_n, ve_hi, flip)
    if _PROGRAM is None:
        _PROGRAM = _build_program(1, ve_hi, flip)
    return _PROGRAM


def make_in_maps(x, W_K, W_Q, W_V, W_O):
    x = np.asarray(x, np.float32)
    W_K = np.asarray(W_K, np.float32)
    W_Q = np.asarray(W_Q, np.float32)
    W_V = np.asarray(W_V, np.float32)
    W_O = np.asarray(W_O, np.float32)

    wqT = np.ascontiguousarray((W_Q.transpose(2, 0, 1).reshape(D, H * DH)) * PRESCALE)
    wkT = np.ascontiguousarray(W_K.transpose(2, 0, 1).reshape(D, H * DH))
    wvT = np.ascontiguousarray(W_V.transpose(2, 0, 1).reshape(D, H * DH))
    wpack = np.concatenate([wqT, wkT, wvT], axis=1).astype(BF16)
    woT_flat = 0.5 * W_O.T  # [f, d']
    woT = np.zeros((2, 128, D), np.float32)
    for p in range(2):
        woT[p, 0:32] = woT_flat[(2 * p) * 32 : (2 * p) * 32 + 32]
        woT[p, 64:96] = woT_flat[(2 * p + 1) * 32 : (2 * p + 1) * 32 + 32]

    in_maps = []
    for core in range(NCORES):
        b, qc = divmod(core, 4)
        xb = x[b]
        xkT_b = np.ascontiguousarray(xb.T).astype(BF16)
        xqT_c = np.ascontiguousarray(xb[qc * QCHUNK : (qc + 1) * QCHUNK].T).astype(BF16)
        # exact blend constant: c = 0.5 * (sum_k v[k]) @ W_O^T, split into
        # bf16 hi + lo for two exact-ish K=1 matmuls
        sv = (xb.sum(0, dtype=np.float64) @ wvT.astype(np.float64))
        c = (0.5 * (sv @ W_O.T.astype(np.float64))).astype(np.float32)
        c_hi = c.astype(BF16)
        c_lo = (c - c_hi.astype(np.float32)).astype(BF16)
        cpack = np.concatenate([c_hi, c_lo])[None, :].astype(BF16)
        in_maps.append(
            {
                "xkT": xkT_b,
                "xqT": xqT_c,
                "wpack": wpack,
                "woT": woT.astype(BF16),
                "cpack": cpack,
            }
        )
    return in_maps


def kernel(x, W_K, W_Q, W_V, W_O):
    from concourse.bass_utils import run_bass_kernel_spmd

    nc = _get_program()
    in_maps = make_in_maps(x, W_K, W_Q, W_V, W_O)
    res = run_bass_kernel_spmd(nc, in_maps, core_ids=list(range(NCORES)))
    full = np.empty((B, S, D), np.float32)
    for core in range(NCORES):
        b, qc = divmod(core, 4)
        full[b, qc * QCHUNK : (qc + 1) * QCHUNK, :] = res.results[core]["out"].T
    return full
